# revision 3
# baseline (speedup 1.0000x reference)
"""Trainium2 Bass kernel for GNN message passing (gather + segment_sum).

reference:
    row, col = edge_index
    out = segment_sum(x[row], col, num_segments=x.shape[0])    # [100000, 128]

Architecture (destination-sharded one-hot-matmul scatter-add, no collectives):
- Host: shard destination nodes contiguously across 8 cores (12500/core).
  Per core, dests are grouped into 98 windows of 128 nodes. Each window has 4
  source-chunk segments (x split into 4 row-quarters of 25000 so dma_gather's
  int16 indices can address them) of 2 tiles (256 edge slots) each -> 8 tiles
  per window, 784 tiles per core. Segment overflow is checked (max observed
  250/256 for this problem's uniform graph); on overflow a host fallback
  computes the answer instead of returning garbage.
- Device (identical SPMD program on 8 cores; only the data differs):
  * dma_gather (ANT extended SWDGE op) pulls 512B x-rows for each edge slot,
    one call per (window-group, source-chunk), spread over 4 SWDGE queues so
    all four Q7 core-pairs generate descriptors in parallel.
  * Per edge tile: one-hot OH[e,d] = (ohpos[e] == iota_d) via VectorE
    tensor_scalar is_equal (padding slots carry ohpos=-1 -> zero row), then
    TensorE matmul psum[d,f] += OH.T @ msg accumulated over the window's 8
    tiles.
  * Per window: PSUM -> SBUF via ScalarE copy, then contiguous 64KB DMA to
    the window's 128 output rows.
- Host: concatenate the 8 per-core output slices.
"""

from dataclasses import dataclass

import numpy as np

import jax
from jax.experimental.shard_map import shard_map
from jax.sharding import Mesh, NamedSharding, PartitionSpec

import concourse.bass as bass
import concourse.mybir as mybir
import concourse.tile as tile
from concourse import bass2jax
from concourse.vector_clock import ScopedClock

# ---------------------------------------------------------------------------
# Toolchain workarounds for this walrus build:
# 1) The ISA here allows at most ONE sync-wait command per instruction
#    ("Too many sync wait commands" at codegen otherwise). TileContext's tail
#    drain carries one wait per live semaphore lane, and the scheduler can
#    attach several waits to body instructions too, so every surplus wait is
#    moved onto its own same-engine NOP placed directly before the original
#    instruction (the sequencer executes them in order — semantics identical).
# 2) Extended/pseudo Pool instructions (load_library, dma_gather) need
#    codegen_inst_isa_subclasses before walrus, else "ISA wrong length".
# ---------------------------------------------------------------------------


def _drain_and_barrier_split(self, tick_clock, wait_clock):
    nc = self.nc
    drain_inst = nc.sync.drain()
    wait_clock.add_sem_waits(
        drain_inst.ins, ScopedClock({None: tick_clock.global_clock})
    )
    si = drain_inst.ins.sync_info
    if si is not None and len(si.on_wait) > 0:
        waits = list(si.on_wait)
        si.on_wait = []
        for w in waits:
            nop = nc.sync.nop(nofuse=True)
            nop.ins.sync_info = mybir.SyncInfo(on_wait=[w], on_update=[])
    nc.all_engine_barrier()
    assert self.sems is not None
    popped = nc._tile_sem_poison_stack.pop()
    assert popped is self._sem_poison
    nc.clear_and_free_semaphores(list(self.sems.allocated().values()))
    nc.all_engine_barrier()


tile.TileContext._drain_and_barrier = _drain_and_barrier_split


def split_multi_waits(nc: "bass.Bass", max_waits: int = 1) -> None:
    k = 0
    for fn in nc.m.functions:
        for bb in fn.blocks:
            il = list(bb.instructions)
            out = []
            changed = False
            for inst in il:
                si = inst.sync_info
                if si is not None and len(si.on_wait) > max_waits:
                    waits = list(si.on_wait)
                    si.on_wait = waits[:max_waits]
                    for w in waits[max_waits:]:
                        nop = mybir.InstNoOp(
                            name=f"I-wsplit-{k}", ins=[], outs=[]
                        )
                        k += 1
                        nop.engine = inst.engine
                        nop.sync_info = mybir.SyncInfo(
                            on_wait=[w], on_update=[]
                        )
                        nc.register_instruction(nop, overwrite=True)
                        out.append(nop)
                        changed = True
                out.append(inst)
            if changed:
                bb.instructions = out


# ---------------------------------------------------------------------------
# Kernel
# ---------------------------------------------------------------------------

D = 128
P = 128
N_CORES = 8
NCHUNK = 4  # x row-quarters (int16 dma_gather index reach)


@dataclass(frozen=True)
class Cfg:
    n_nodes: int
    node_per_core: int
    seg_tiles: int  # tiles per (window, chunk) segment
    group_w: int  # windows per gather group
    gbufs: int = 2

    @property
    def chunk_rows(self) -> int:
        return self.n_nodes // NCHUNK

    @property
    def W(self) -> int:
        return -(-self.node_per_core // P)

    @property
    def T(self) -> int:
        return NCHUNK * self.seg_tiles

    @property
    def NT(self) -> int:
        return self.W * self.T

    @property
    def region_cols(self) -> int:
        return self.W * self.seg_tiles

    @property
    def n_groups(self) -> int:
        assert self.W % self.group_w == 0
        return self.W // self.group_w

    @property
    def call_tiles(self) -> int:
        return self.group_w * self.seg_tiles

    @property
    def call_idxs(self) -> int:
        return self.call_tiles * P

    @property
    def idx_cols(self) -> int:
        return self.n_groups * (self.call_idxs // 16)

    @property
    def out_rows(self) -> int:
        return self.W * P

    @property
    def cap(self) -> int:
        return self.seg_tiles * P


CFG = Cfg(n_nodes=100000, node_per_core=12500, seg_tiles=2, group_w=7,
          gbufs=2)


def build(cfg: Cfg, reps: int = 1) -> bass.Bass:
    from concourse.library_config import mlp
    from concourse.library_overlay import lower_extended_insts

    assert cfg.chunk_rows <= 32767
    nc = bass.Bass(num_swdge_queues=NCHUNK, dynamic_dma_scratch_size=65536)
    x = nc.declare_dram_parameter("x", [cfg.n_nodes, D], mybir.dt.float32,
                                  isOutput=False)
    idxs = nc.declare_dram_parameter(
        "idxs", [P, NCHUNK * cfg.idx_cols], mybir.dt.int16, isOutput=False)
    ohpos = nc.declare_dram_parameter("ohpos", [P, cfg.NT], mybir.dt.float32,
                                      isOutput=False)
    iota = nc.declare_dram_parameter("iota128", [P, P], mybir.dt.float32,
                                     isOutput=False)
    out = nc.declare_dram_parameter("out", [cfg.out_rows, D],
                                    mybir.dt.float32, isOutput=True)

    CT = cfg.call_tiles
    nc.gpsimd.load_library(mlp)
    with tile.TileContext(nc) as tc:
        with (
            tc.tile_pool(name="tabs", bufs=1) as tabs,
            tc.tile_pool(name="gbuf", bufs=cfg.gbufs) as gbuf,
            tc.tile_pool(name="ohb", bufs=4) as ohb,
            tc.tile_pool(name="psumb", bufs=4, space="PSUM") as psumb,
            tc.tile_pool(name="outb", bufs=4) as outb,
        ):
            idxs_sb = tabs.tile([P, NCHUNK * cfg.idx_cols], mybir.dt.int16)
            ohpos_sb = tabs.tile([P, cfg.NT], mybir.dt.float32)
            iota_sb = tabs.tile([P, P], mybir.dt.float32)
            nc.sync.dma_start(out=idxs_sb[:], in_=idxs[:])
            nc.sync.dma_start(out=ohpos_sb[:], in_=ohpos[:])
            nc.sync.dma_start(out=iota_sb[:], in_=iota[:])

            # one shared register for the gather count — a fresh to_reg per
            # call exhausts the Pool register file at this call count
            nidx_reg = nc.gpsimd.to_reg(cfg.call_idxs)

            for g in range(cfg.n_groups * reps):
                g = g % cfg.n_groups
                chts = []
                for r in range(NCHUNK):
                    ch = gbuf.tile([P, CT * D], mybir.dt.float32,
                                   tag=f"ch{r}")
                    c0 = r * cfg.idx_cols + g * (cfg.call_idxs // 16)
                    nc.gpsimd.dma_gather(
                        ch[:].rearrange("p (c d) -> p c d", c=CT),
                        x[r * cfg.chunk_rows:(r + 1) * cfg.chunk_rows, :],
                        idxs_sb[:, c0:c0 + cfg.call_idxs // 16],
                        cfg.call_idxs,
                        nidx_reg,
                        D,
                        queue_num=r,
                        # single_packet coalesces the whole descriptor stream
                        # into one SDMA packet; beyond ~1K descriptors that
                        # wedges the DMA engine (HW hang). Multi-packet is
                        # required at this call size.
                        single_packet=False,
                    )
                    chts.append(ch)
                for wl in range(cfg.group_w):
                    w = g * cfg.group_w + wl
                    ps = psumb.tile([P, D], mybir.dt.float32, tag="ps")
                    k = 0
                    last = cfg.T - 1
                    for r in range(NCHUNK):
                        for i in range(cfg.seg_tiles):
                            tcol = (r * cfg.region_cols
                                    + cfg.seg_tiles * w + i)
                            lcol = cfg.seg_tiles * wl + i
                            oh = ohb.tile([P, P], mybir.dt.float32, tag="oh")
                            nc.vector.tensor_scalar(
                                out=oh[:],
                                in0=iota_sb[:],
                                scalar1=ohpos_sb[:, tcol:tcol + 1],
                                scalar2=None,
                                op0=mybir.AluOpType.is_equal,
                            )
                            nc.tensor.matmul(
                                ps[:],
                                lhsT=oh[:],
                                rhs=chts[r][:, lcol * D:(lcol + 1) * D],
                                start=(k == 0),
                                stop=(k == last),
                            )
                            k += 1
                    ob = outb.tile([P, D], mybir.dt.float32, tag="ob")
                    nc.scalar.copy(out=ob[:], in_=ps[:])
                    nc.sync.dma_start(
                        out=out[w * P:(w + 1) * P, :], in_=ob[:]
                    )
    split_multi_waits(nc)
    lower_extended_insts(nc)
    return nc


def prep_core(row, col, node_base, cfg: Cfg):
    """Slot assignment for one core. Returns (idxs int16, ohpos f32)."""
    lo, hi = node_base, node_base + cfg.node_per_core
    m = (col >= lo) & (col < hi)
    lcol = (col[m] - lo).astype(np.int64)
    lrow = row[m].astype(np.int64)

    w = lcol >> 7
    pos = lcol & 127
    ck = lrow // cfg.chunk_rows
    lidx = lrow - ck * cfg.chunk_rows

    key = w * NCHUNK + ck
    order = np.argsort(key, kind="stable")
    key_s = key[order]
    pos_s = pos[order]
    lidx_s = lidx[order]

    nseg = cfg.W * NCHUNK
    counts = np.bincount(key_s, minlength=nseg)
    if counts.max(initial=0) > cfg.cap:
        raise ValueError(
            f"segment overflow: {counts.max()} > {cfg.cap}"
        )
    starts = np.zeros(nseg, np.int64)
    np.cumsum(counts[:-1], out=starts[1:])
    rank = np.arange(len(key_s)) - starts[key_s]

    w_e = key_s // NCHUNK
    r_e = key_s % NCHUNK
    tcol = r_e * cfg.region_cols + w_e * cfg.seg_tiles + (rank >> 7)
    lane = rank & 127

    srcidx = np.zeros((P, cfg.NT), np.int16)
    ohpos = np.full((P, cfg.NT), -1.0, np.float32)
    srcidx[lane, tcol] = lidx_s
    ohpos[lane, tcol] = pos_s

    # per (chunk, group) wrapped int16 index blocks: call order i = j*128+p,
    # wrapped in 16 partitions and replicated 8x (what the Q7 pairs read)
    idxs = np.zeros((P, NCHUNK * cfg.idx_cols), np.int16)
    CT = cfg.call_tiles
    ci16 = cfg.call_idxs // 16
    for r in range(NCHUNK):
        for g in range(cfg.n_groups):
            cols = r * cfg.region_cols + g * CT + np.arange(CT)
            flat = srcidx[:, cols].T.reshape(-1)
            wrapped = flat.reshape(ci16, 16).T
            c0 = r * cfg.idx_cols + g * ci16
            idxs[:, c0:c0 + ci16] = np.tile(wrapped, (8, 1))
    return idxs, ohpos


def prep_all(x, edge_index, cfg: Cfg):
    row = np.asarray(edge_index[0])
    col = np.asarray(edge_index[1])
    xf = np.ascontiguousarray(np.asarray(x, dtype=np.float32))
    it = np.tile(np.arange(P, dtype=np.float32), (P, 1))
    in_maps = []
    for c in range(N_CORES):
        idxs, ohpos = prep_core(row, col, c * cfg.node_per_core, cfg)
        in_maps.append({"x": xf, "idxs": idxs, "ohpos": ohpos,
                        "iota128": it})
    return in_maps


class SpmdRunner:
    """PJRT SPMD runner for a prebuilt Bass module.

    Mirrors bass2jax.run_bass_via_pjrt but stages inputs with per-device
    device_put + make_array_from_single_device_arrays and reads outputs
    shard-by-shard: no host<->global-array slicing ops get compiled (this
    toolchain's penguin DataLocalityOpt rejects them for large arrays).
    """

    def __init__(self, nc: bass.Bass, n_cores: int = N_CORES):
        bass2jax.install_neuronx_cc_hook()
        self.nc = nc
        self.n_cores = n_cores
        pname = nc.partition_id_tensor.name if nc.partition_id_tensor else None
        self.partition_name = pname
        in_names, out_names, out_avals = [], [], []
        for alloc in nc.m.functions[0].allocations:
            if not isinstance(alloc, mybir.MemoryLocationSet):
                continue
            name = alloc.memorylocations[0].name
            if alloc.kind == "ExternalInput":
                if name != pname:
                    in_names.append(name)
            elif alloc.kind == "ExternalOutput":
                out_names.append(name)
                out_avals.append(
                    jax.core.ShapedArray(
                        tuple(alloc.tensor_shape), mybir.dt.np(alloc.dtype)
                    )
                )
        self.in_names = in_names
        self.out_names = out_names
        self.out_avals = out_avals
        self.devices = jax.devices()[:n_cores]
        self.mesh = Mesh(np.asarray(self.devices), ("core",))
        self.sharding = NamedSharding(self.mesh, PartitionSpec("core"))
        all_in_names = list(in_names) + list(out_names)
        if pname is not None:
            all_in_names.append(pname)

        def _body(*args):
            operands = list(args)
            if pname is not None:
                operands.append(bass2jax.partition_id_tensor())
            return tuple(
                bass2jax._bass_exec_p.bind(
                    *operands,
                    out_avals=tuple(out_avals),
                    in_names=tuple(all_in_names),
                    out_names=tuple(out_names),
                    lowering_input_output_aliases=(),
                    sim_require_finite=True,
                    sim_require_nnan=True,
                    nc=nc,
                )
            )

        n_args = len(in_names) + len(out_names)
        self.fn = jax.jit(
            shard_map(
                _body,
                mesh=self.mesh,
                in_specs=(PartitionSpec("core"),) * n_args,
                out_specs=(PartitionSpec("core"),) * len(out_names),
                check_rep=False,
            ),
            keep_unused=True,
        )

    def _global(self, per_core_arrays):
        shape = per_core_arrays[0].shape
        gshape = (self.n_cores * shape[0],) + tuple(shape[1:])
        bufs = [
            jax.device_put(a, d)
            for a, d in zip(per_core_arrays, self.devices)
        ]
        return jax.make_array_from_single_device_arrays(
            gshape, self.sharding, bufs
        )

    def stage(self, in_maps):
        args = [
            self._global([np.asarray(m[name]) for m in in_maps])
            for name in self.in_names
        ]
        args += [
            self._global(
                [np.zeros(av.shape, av.dtype) for _ in range(self.n_cores)]
            )
            for av in self.out_avals
        ]
        return args

    def run(self, args):
        outs = self.fn(*args)
        jax.block_until_ready(outs)
        return outs

    def to_numpy(self, outs):
        res = [dict() for _ in range(self.n_cores)]
        for i, name in enumerate(self.out_names):
            shards = sorted(
                outs[i].addressable_shards,
                key=lambda s: s.index[0].start or 0,
            )
            assert len(shards) == self.n_cores
            for c, s in enumerate(shards):
                res[c][name] = np.asarray(s.data)
        return res

    def __call__(self, in_maps):
        return self.to_numpy(self.run(self.stage(in_maps)))


_NC_CACHE = {}
_RUNNER_CACHE = {}


def _get_nc(cfg: Cfg) -> bass.Bass:
    nc = _NC_CACHE.get(cfg)
    if nc is None:
        nc = build(cfg)
        _NC_CACHE[cfg] = nc
    return nc


def _get_runner(cfg: Cfg) -> SpmdRunner:
    r = _RUNNER_CACHE.get(cfg)
    if r is None:
        r = SpmdRunner(_get_nc(cfg))
        _RUNNER_CACHE[cfg] = r
    return r


def kernel(x: np.ndarray, edge_index: np.ndarray) -> np.ndarray:
    x = np.asarray(x)
    edge_index = np.asarray(edge_index)
    try:
        in_maps = prep_all(x, edge_index, CFG)
    except ValueError:
        # Segment-capacity overflow (an edge distribution far from this
        # problem's uniform random graph): fall back to a host computation
        # rather than returning wrong results.
        out = np.zeros((x.shape[0], x.shape[1]), np.float32)
        np.add.at(
            out,
            np.asarray(edge_index[1], np.int64),
            np.asarray(x, np.float32)[np.asarray(edge_index[0], np.int64)],
        )
        return out
    res = _get_runner(CFG)(in_maps)
    return np.concatenate(
        [res[c]["out"][: CFG.node_per_core] for c in range(N_CORES)]
    )



# revision 26
# speedup vs baseline: 2.4443x; 2.4443x over previous
"""Trainium2 Bass kernel for GNN message passing (gather + segment_sum).

reference:
    row, col = edge_index
    out = segment_sum(x[row], col, num_segments=x.shape[0])    # [100000, 128]

Architecture (destination-sharded one-hot-matmul scatter-add, no collectives):
- Host: shard destination nodes contiguously across 8 cores (12500/core).
  Per core, dests are grouped into 98 windows of 128 nodes. Each window has 4
  source-chunk segments (x split into 4 row-quarters of 25000 so dma_gather's
  int16 indices can address them) of 2 tiles (256 edge slots) each -> 8 tiles
  per window, 784 tiles per core. Segment overflow is checked (max observed
  250/256 for this problem's uniform graph); on overflow a host fallback
  computes the answer instead of returning garbage.
- Device (identical SPMD program on 8 cores; only the data differs):
  * Messages, one-hots, and matmuls run in bf16 (CFG.bf16): host casts x to
    bf16 (256B rows), halving HBM gather payload; PE runs 1 cycle/row vs 4
    for fp32, and the one-hot is_equal gets the 4x DVE mode. PSUM
    accumulates fp32, so only the input rounding (~2^-9) is lost; measured
    rel err 1.9e-03 vs the 2e-02 gate.
  * dma_gather (ANT extended SWDGE op) pulls x-rows for each edge slot,
    one call per (window-group, source-chunk), spread over 4 SWDGE queues so
    all four Q7 core-pairs generate descriptors in parallel. gbufs=4 gather
    buffers per chunk keep the Pool/DMA pipeline 3 groups ahead of compute.
  * Per edge tile: one-hot OH[e,d] = (ohpos[e] == iota_d) via VectorE
    tensor_scalar is_equal (padding slots carry ohpos=-1 -> zero row), then
    TensorE matmul psum[d,f] += OH.T @ msg accumulated over the window's 8
    tiles.
  * Per window: PSUM -> a persistent SBUF output buffer via ScalarE copy
    (CFG.bigout); one strided DMA per 7-window group flushes to HBM, so no
    per-window DMA ever gates PSUM recycling behind the gather's descriptor
    queue on the shared DMA engines.
- Host: concatenate the 8 per-core output slices.

An alternative exact-count layout (prep_v2/build_v2, CFG.v2) packs edges
contiguously per (group, chunk) with per-core valid counts fed to the
gather via Pool reg_load, cutting descriptors 100352 -> 78125/core; tiles
may span two windows, handled by a 256-wide one-hot over iota256 with
compare value pos + 128*k. It validates (rel err 1.9e-03) but measured no
faster end-to-end on this toolchain, so the static v1 layout ships.
"""

from dataclasses import dataclass

import numpy as np

import jax
from jax.experimental.shard_map import shard_map
from jax.sharding import Mesh, NamedSharding, PartitionSpec

import concourse.bass as bass
import concourse.mybir as mybir
import concourse.tile as tile
from concourse import bass2jax
from concourse.vector_clock import ScopedClock

# ---------------------------------------------------------------------------
# Toolchain workarounds for this walrus build:
# 1) The ISA here allows at most ONE sync-wait command per instruction
#    ("Too many sync wait commands" at codegen otherwise). TileContext's tail
#    drain carries one wait per live semaphore lane, and the scheduler can
#    attach several waits to body instructions too, so every surplus wait is
#    moved onto its own same-engine NOP placed directly before the original
#    instruction (the sequencer executes them in order — semantics identical).
# 2) Extended/pseudo Pool instructions (load_library, dma_gather) need
#    codegen_inst_isa_subclasses before walrus, else "ISA wrong length".
# ---------------------------------------------------------------------------


def _drain_and_barrier_split(self, tick_clock, wait_clock):
    nc = self.nc
    drain_inst = nc.sync.drain()
    wait_clock.add_sem_waits(
        drain_inst.ins, ScopedClock({None: tick_clock.global_clock})
    )
    si = drain_inst.ins.sync_info
    if si is not None and len(si.on_wait) > 0:
        waits = list(si.on_wait)
        si.on_wait = []
        for w in waits:
            nop = nc.sync.nop(nofuse=True)
            nop.ins.sync_info = mybir.SyncInfo(on_wait=[w], on_update=[])
    nc.all_engine_barrier()
    assert self.sems is not None
    popped = nc._tile_sem_poison_stack.pop()
    assert popped is self._sem_poison
    nc.clear_and_free_semaphores(list(self.sems.allocated().values()))
    nc.all_engine_barrier()


tile.TileContext._drain_and_barrier = _drain_and_barrier_split


def split_multi_waits(nc: "bass.Bass", max_waits: int = 1) -> None:
    k = 0
    for fn in nc.m.functions:
        for bb in fn.blocks:
            il = list(bb.instructions)
            out = []
            changed = False
            for inst in il:
                si = inst.sync_info
                if si is not None and len(si.on_wait) > max_waits:
                    waits = list(si.on_wait)
                    si.on_wait = waits[:max_waits]
                    for w in waits[max_waits:]:
                        nop = mybir.InstNoOp(
                            name=f"I-wsplit-{k}", ins=[], outs=[]
                        )
                        k += 1
                        nop.engine = inst.engine
                        nop.sync_info = mybir.SyncInfo(
                            on_wait=[w], on_update=[]
                        )
                        nc.register_instruction(nop, overwrite=True)
                        out.append(nop)
                        changed = True
                out.append(inst)
            if changed:
                bb.instructions = out


# ---------------------------------------------------------------------------
# Kernel
# ---------------------------------------------------------------------------

D = 128
P = 128
N_CORES = 8
NCHUNK = 4  # x row-quarters (int16 dma_gather index reach)


@dataclass(frozen=True)
class Cfg:
    n_nodes: int
    node_per_core: int
    seg_tiles: int  # tiles per (window, chunk) segment
    group_w: int  # windows per gather group
    gbufs: int = 2
    bf16: bool = False  # gather/one-hot/matmul in bf16 (PSUM stays f32)
    scratch: int = 65536  # SWDGE descriptor-ring carveout bytes
    bigout: bool = False  # accumulate output in SBUF, flush per group
    psum_bufs: int = 4
    v2: bool = False  # exact-count packed layout (build_v2/prep_v2)
    phases: int = 0  # >0: serialize gather/compute in this many phases

    @property
    def chunk_rows(self) -> int:
        return self.n_nodes // NCHUNK

    @property
    def W(self) -> int:
        return -(-self.node_per_core // P)

    @property
    def T(self) -> int:
        return NCHUNK * self.seg_tiles

    @property
    def NT(self) -> int:
        return self.W * self.T

    @property
    def region_cols(self) -> int:
        return self.W * self.seg_tiles

    @property
    def n_groups(self) -> int:
        assert self.W % self.group_w == 0
        return self.W // self.group_w

    @property
    def call_tiles(self) -> int:
        return self.group_w * self.seg_tiles

    @property
    def call_idxs(self) -> int:
        return self.call_tiles * P

    @property
    def idx_cols(self) -> int:
        return self.n_groups * (self.call_idxs // 16)

    @property
    def out_rows(self) -> int:
        return self.W * P

    @property
    def cap(self) -> int:
        return self.seg_tiles * P


CFG = Cfg(n_nodes=100000, node_per_core=12500, seg_tiles=2, group_w=7,
          gbufs=4, bf16=True, bigout=True, psum_bufs=8)


def build(cfg: Cfg, reps: int = 1) -> bass.Bass:
    from concourse.library_config import mlp
    from concourse.library_overlay import lower_extended_insts

    assert cfg.chunk_rows <= 32767
    mdt = mybir.dt.bfloat16 if cfg.bf16 else mybir.dt.float32
    nc = bass.Bass(num_swdge_queues=NCHUNK,
                   dynamic_dma_scratch_size=cfg.scratch)
    x = nc.declare_dram_parameter("x", [cfg.n_nodes, D], mdt,
                                  isOutput=False)
    idxs = nc.declare_dram_parameter(
        "idxs", [P, NCHUNK * cfg.idx_cols], mybir.dt.int16, isOutput=False)
    ohpos = nc.declare_dram_parameter("ohpos", [P, cfg.NT], mybir.dt.float32,
                                      isOutput=False)
    iota = nc.declare_dram_parameter("iota128", [P, P], mdt,
                                     isOutput=False)
    out = nc.declare_dram_parameter("out", [cfg.out_rows, D],
                                    mybir.dt.float32, isOutput=True)

    CT = cfg.call_tiles
    nc.gpsimd.load_library(mlp)
    with tile.TileContext(nc) as tc:
        with (
            tc.tile_pool(name="tabs", bufs=1) as tabs,
            tc.tile_pool(name="gbuf", bufs=cfg.gbufs) as gbuf,
            tc.tile_pool(name="ohb", bufs=4) as ohb,
            tc.tile_pool(name="psumb", bufs=cfg.psum_bufs,
                         space="PSUM") as psumb,
            tc.tile_pool(name="outb", bufs=4) as outb,
        ):
            idxs_sb = tabs.tile([P, NCHUNK * cfg.idx_cols], mybir.dt.int16)
            ohpos_sb = tabs.tile([P, cfg.NT], mybir.dt.float32)
            iota_sb = tabs.tile([P, P], mdt)
            nc.sync.dma_start(out=idxs_sb[:], in_=idxs[:])
            nc.sync.dma_start(out=ohpos_sb[:], in_=ohpos[:])
            nc.sync.dma_start(out=iota_sb[:], in_=iota[:])
            outsb = (tabs.tile([P, cfg.W * D], mybir.dt.float32,
                                name="outsb", tag="outsb")
                     if cfg.bigout else None)

            # one shared register for the gather count — a fresh to_reg per
            # call exhausts the Pool register file at this call count
            nidx_reg = nc.gpsimd.to_reg(cfg.call_idxs)

            for g in range(cfg.n_groups * reps):
                g = g % cfg.n_groups
                chts = []
                for r in range(NCHUNK):
                    ch = gbuf.tile([P, CT * D], mdt,
                                   tag=f"ch{r}")
                    c0 = r * cfg.idx_cols + g * (cfg.call_idxs // 16)
                    nc.gpsimd.dma_gather(
                        ch[:].rearrange("p (c d) -> p c d", c=CT),
                        x[r * cfg.chunk_rows:(r + 1) * cfg.chunk_rows, :],
                        idxs_sb[:, c0:c0 + cfg.call_idxs // 16],
                        cfg.call_idxs,
                        nidx_reg,
                        D,
                        queue_num=r,
                        # single_packet coalesces the whole descriptor stream
                        # into one SDMA packet; beyond ~1K descriptors that
                        # wedges the DMA engine (HW hang). Multi-packet is
                        # required at this call size.
                        single_packet=False,
                    )
                    chts.append(ch)
                for wl in range(cfg.group_w):
                    w = g * cfg.group_w + wl
                    ps = psumb.tile([P, D], mybir.dt.float32, tag="ps")
                    k = 0
                    last = cfg.T - 1
                    for r in range(NCHUNK):
                        for i in range(cfg.seg_tiles):
                            tcol = (r * cfg.region_cols
                                    + cfg.seg_tiles * w + i)
                            lcol = cfg.seg_tiles * wl + i
                            oh = ohb.tile([P, P], mdt, tag="oh")
                            nc.vector.tensor_scalar(
                                out=oh[:],
                                in0=iota_sb[:],
                                scalar1=ohpos_sb[:, tcol:tcol + 1],
                                scalar2=None,
                                op0=mybir.AluOpType.is_equal,
                            )
                            nc.tensor.matmul(
                                ps[:],
                                lhsT=oh[:],
                                rhs=chts[r][:, lcol * D:(lcol + 1) * D],
                                start=(k == 0),
                                stop=(k == last),
                            )
                            k += 1
                    if cfg.bigout:
                        nc.scalar.copy(out=outsb[:, w * D:(w + 1) * D],
                                       in_=ps[:])
                        if wl == cfg.group_w - 1:
                            w0 = g * cfg.group_w
                            nc.sync.dma_start(
                                out=out[w0 * P:(w + 1) * P, :]
                                .rearrange("(w p) d -> p w d",
                                           w=cfg.group_w),
                                in_=outsb[:, w0 * D:(w + 1) * D]
                                .rearrange("p (w d) -> p w d",
                                           w=cfg.group_w),
                            )
                    else:
                        ob = outb.tile([P, D], mybir.dt.float32, tag="ob")
                        nc.scalar.copy(out=ob[:], in_=ps[:])
                        nc.sync.dma_start(
                            out=out[w * P:(w + 1) * P, :], in_=ob[:]
                        )
    split_multi_waits(nc)
    lower_extended_insts(nc)
    return nc


# ---------------------------------------------------------------------------
# v2 layout: exact-count packed gather (no intra-segment padding)
#
# Edges are packed contiguously per (group, chunk) sorted by window; gather
# calls carry per-core valid counts in a Pool register (reg_load), so padding
# slots generate no DMA descriptors.  A tile may span up to two consecutive
# windows; the per-tile window list (union across all 8 cores, so one shared
# SPMD program works) drives one wide one-hot per <=2 windows: OH built over
# iota256 with compare value pos + 128*k, then one matmul per 128-col slice.
# ---------------------------------------------------------------------------


def prep_v2(x, edge_index, cfg: Cfg):
    row = np.asarray(edge_index[0]).astype(np.int64)
    col = np.asarray(edge_index[1]).astype(np.int64)
    mdt_np = mybir.dt.np(mybir.dt.bfloat16 if cfg.bf16 else mybir.dt.float32)
    xf = np.ascontiguousarray(np.asarray(x, dtype=np.float32).astype(mdt_np))
    NG, GW = cfg.n_groups, cfg.group_w

    cores = []
    wcounts = np.zeros((N_CORES, NG, NCHUNK, GW), np.int64)
    for c in range(N_CORES):
        lo = c * cfg.node_per_core
        m = (col >= lo) & (col < lo + cfg.node_per_core)
        lcol = col[m] - lo
        lrow = row[m]
        w = lcol >> 7
        pos = lcol & 127
        ck = lrow // cfg.chunk_rows
        lidx = lrow - ck * cfg.chunk_rows
        g = w // GW
        wl = w % GW
        order = np.lexsort((wl, ck, g))
        cores.append((g[order], ck[order], wl[order], pos[order],
                      lidx[order]))
        np.add.at(wcounts[c], (g[order], ck[order], wl[order]), 1)

    C = wcounts.sum(axis=3)  # [core, g, ck]
    if C.min() < 1:
        raise ValueError("v2 layout needs >=1 edge per (core, group, chunk)")
    T = -(-C.max(axis=0) // P)  # [g, ck] static tile counts

    winlists = []  # [g][ck] -> tuple per tile of sorted wl tuple
    for g in range(NG):
        row_l = []
        for ck in range(NCHUNK):
            tiles = [set() for _ in range(int(T[g, ck]))]
            for c in range(N_CORES):
                posn = 0
                for wl in range(GW):
                    n = int(wcounts[c, g, ck, wl])
                    if n == 0:
                        continue
                    for t in range(posn // P, (posn + n - 1) // P + 1):
                        tiles[t].add(wl)
                    posn += n
            row_l.append(tuple(tuple(sorted(s)) for s in tiles))
        winlists.append(tuple(row_l))
    Cmin = C.min(axis=0)  # [g, ck] min valid count across cores
    layout = (tuple(tuple(int(v) for v in T[g]) for g in range(NG)),
              tuple(winlists),
              tuple(tuple(int(v) for v in Cmin[g]) for g in range(NG)))

    # static oh-instruction schedule: per (g, ck, tile) split winlist into
    # chunks of <=2; each chunk is one ohpos column
    n_oh = 0
    for g in range(NG):
        for ck in range(NCHUNK):
            for wl_list in winlists[g][ck]:
                n_oh += max(1, -(-len(wl_list) // 2))
    Tmax = int(T.max())
    ci16_total = int(T.sum()) * 8

    in_maps = []
    for c in range(N_CORES):
        gs, cks, wls, poss, lidxs = cores[c]
        idxs = np.zeros((16, ci16_total), np.int16)
        ohpos = np.full((P, n_oh), -1.0, np.float32)
        counts = np.zeros((P, NG * NCHUNK), np.int32)
        base = 0
        ohcol = 0
        e0 = 0
        # edges are sorted by (g, ck, wl); walk segments in order
        for g in range(NG):
            for ck in range(NCHUNK):
                n = int(C[c, g, ck])
                tcount = int(T[g, ck])
                nslots = tcount * P
                sl = slice(e0, e0 + n)
                stream = np.full(nslots, -1, np.int16)
                stream[:n] = lidxs[sl]
                counts[:, g * NCHUNK + ck] = n
                idxs[:, base:base + tcount * 8] = (
                    stream.reshape(tcount * 8, 16).T
                )
                base += tcount * 8
                # oh columns for this segment
                wl_seg = wls[sl]
                pos_seg = poss[sl]
                for t, wl_list in enumerate(winlists[g][ck]):
                    s0, s1 = t * P, min((t + 1) * P, nslots)
                    nvals = max(0, min(s1, n) - s0)
                    lanes = np.arange(s0, s0 + nvals) - s0
                    for pair_i in range(max(1, -(-len(wl_list) // 2))):
                        pair = wl_list[2 * pair_i:2 * pair_i + 2]
                        if nvals > 0:
                            wl_t = wl_seg[s0:s0 + nvals]
                            pos_t = pos_seg[s0:s0 + nvals]
                            for k, wl in enumerate(pair):
                                mk = wl_t == wl
                                ohpos[lanes[mk], ohcol] = (
                                    pos_t[mk] + 128 * k
                                )
                        ohcol += 1
                e0 += n
        assert ohcol == n_oh and base == ci16_total
        it = np.tile(np.arange(256, dtype=np.float32), (P, 1)).astype(mdt_np)
        in_maps.append({"x": xf, "idxs": np.tile(idxs, (8, 1)),
                        "ohpos": ohpos, "iota256": it,
                        "counts": counts})
    return layout, in_maps


def build_v2(cfg: Cfg, layout, reps: int = 1) -> bass.Bass:
    from concourse.library_config import mlp
    from concourse.library_overlay import lower_extended_insts

    T, winlists, Cmin = layout
    NG, GW = cfg.n_groups, cfg.group_w
    Tmax = max(max(r) for r in T)
    ci16_total = sum(sum(r) for r in T) * 8
    n_oh = sum(max(1, -(-len(wl) // 2))
               for g in range(NG) for ck in range(NCHUNK)
               for wl in winlists[g][ck])
    mdt = mybir.dt.bfloat16 if cfg.bf16 else mybir.dt.float32

    nc = bass.Bass(num_swdge_queues=NCHUNK,
                   dynamic_dma_scratch_size=cfg.scratch)
    x = nc.declare_dram_parameter("x", [cfg.n_nodes, D], mdt, isOutput=False)
    idxs = nc.declare_dram_parameter("idxs", [P, ci16_total], mybir.dt.int16,
                                     isOutput=False)
    ohpos = nc.declare_dram_parameter("ohpos", [P, n_oh], mybir.dt.float32,
                                      isOutput=False)
    iota = nc.declare_dram_parameter("iota256", [P, 2 * P], mdt,
                                     isOutput=False)
    counts = nc.declare_dram_parameter("counts", [P, NG * NCHUNK],
                                       mybir.dt.int32, isOutput=False)
    out = nc.declare_dram_parameter("out", [cfg.out_rows, D],
                                    mybir.dt.float32, isOutput=True)

    nc.gpsimd.load_library(mlp)
    with tile.TileContext(nc) as tc:
        with (
            tc.tile_pool(name="tabs", bufs=1) as tabs,
            tc.tile_pool(name="gbuf", bufs=cfg.gbufs) as gbuf,
            tc.tile_pool(name="ohb", bufs=6) as ohb,
            tc.tile_pool(name="psumb", bufs=1, space="PSUM") as psumb,
        ):
            idxs_sb = tabs.tile([P, ci16_total], mybir.dt.int16)
            ohpos_sb = tabs.tile([P, n_oh], mybir.dt.float32)
            iota_sb = tabs.tile([P, 2 * P], mdt)
            counts_sb = tabs.tile([P, NG * NCHUNK], mybir.dt.int32)
            outsb = tabs.tile([P, cfg.W * D], mybir.dt.float32,
                              name="outsb", tag="outsb")
            nc.sync.dma_start(out=idxs_sb[:], in_=idxs[:])
            nc.sync.dma_start(out=ohpos_sb[:], in_=ohpos[:])
            nc.sync.dma_start(out=iota_sb[:], in_=iota[:])
            nc.sync.dma_start(out=counts_sb[:], in_=counts[:])

            cnt_reg = nc.gpsimd.to_reg(0)

            # static idx base offsets per (g, ck)
            bases = {}
            b = 0
            for g in range(NG):
                for ck in range(NCHUNK):
                    bases[(g, ck)] = b
                    b += T[g][ck] * 8

            # per-group matmul totals per window for start/stop flags
            mm_per_wl = []
            for g in range(NG):
                cnt = {wl: 0 for wl in range(GW)}
                for ck in range(NCHUNK):
                    for wl_list in winlists[g][ck]:
                        for wl in wl_list:
                            cnt[wl] += 1
                mm_per_wl.append(cnt)

            if cfg.phases:
                ppg = -(-NG // cfg.phases)  # groups per phase
                phase_groups = [list(range(p0, min(p0 + ppg, NG)))
                                for p0 in range(0, NG, ppg)]
            else:
                phase_groups = [[g] for g in range(NG)]
            all_chts = {}
            for pseq in range(len(phase_groups) * reps):
                glist = phase_groups[pseq % len(phase_groups)]
                for g in glist:
                    all_chts[g] = {}
                    chts = all_chts[g]
                    for ck in range(NCHUNK):
                        tcount = T[g][ck]
                        ch = gbuf.tile([P, Tmax * D], mdt, tag=f"ch{ck}")
                        c0 = bases[(g, ck)]
                        # zero the tiles past every core's valid count: the
                        # gather skips trailing -1 slots, and 0 * one-hot-0
                        # keeps them out of the sums (NaN-safe on fresh SBUF)
                        t0 = Cmin[g][ck] // P
                        nc.vector.memset(ch[:, t0 * D:tcount * D], 0.0)
                        nc.gpsimd.reg_load(
                            cnt_reg,
                            counts_sb[0:1,
                                      g * NCHUNK + ck:g * NCHUNK + ck + 1],
                        )
                        nc.gpsimd.dma_gather(
                            ch[:, :tcount * D].rearrange(
                                "p (c d) -> p c d", c=tcount),
                            x[ck * cfg.chunk_rows:
                              (ck + 1) * cfg.chunk_rows, :],
                            idxs_sb[:, c0:c0 + tcount * 8],
                            tcount * P,
                            cnt_reg,
                            D,
                            queue_num=ck,
                            single_packet=False,
                        )
                        chts[ck] = ch
                for g in (reversed(glist) if cfg.phases else glist):
                    chts = all_chts[g]
                    pss = {wl: psumb.tile([P, D], mybir.dt.float32,
                                          name=f"ps{wl}", tag=f"ps{wl}")
                           for wl in range(GW)}
                    seen = {wl: 0 for wl in range(GW)}
                    ohcol = sum(
                        max(1, -(-len(wl_list) // 2))
                        for gg in range(g)
                        for ck in range(NCHUNK)
                        for wl_list in winlists[gg][ck]
                    )
                    for ck in range(NCHUNK):
                        for t, wl_list in enumerate(winlists[g][ck]):
                            npair = max(1, -(-len(wl_list) // 2))
                            for pair_i in range(npair):
                                pair = wl_list[2 * pair_i:2 * pair_i + 2]
                                width = P * max(1, len(pair))
                                oh = ohb.tile([P, width], mdt,
                                              tag=f"oh{len(pair)}")
                                nc.vector.tensor_scalar(
                                    out=oh[:],
                                    in0=iota_sb[:, :width],
                                    scalar1=ohpos_sb[:, ohcol:ohcol + 1],
                                    scalar2=None,
                                    op0=mybir.AluOpType.is_equal,
                                )
                                for k, wl in enumerate(pair):
                                    seen[wl] += 1
                                    nc.tensor.matmul(
                                        pss[wl][:],
                                        lhsT=oh[:, k * P:(k + 1) * P],
                                        rhs=chts[ck][:, t * D:(t + 1) * D],
                                        start=(seen[wl] == 1),
                                        stop=(seen[wl] == mm_per_wl[g][wl]),
                                    )
                                ohcol += 1
                    for wl in range(GW):
                        w = g * GW + wl
                        nc.scalar.copy(out=outsb[:, w * D:(w + 1) * D],
                                       in_=pss[wl][:])
                    w0 = g * GW
                    nc.sync.dma_start(
                        out=out[w0 * P:(w0 + GW) * P, :]
                        .rearrange("(w p) d -> p w d", w=GW),
                        in_=outsb[:, w0 * D:(w0 + GW) * D]
                        .rearrange("p (w d) -> p w d", w=GW),
                    )
    split_multi_waits(nc)
    lower_extended_insts(nc)
    return nc


def prep_core(row, col, node_base, cfg: Cfg):
    """Slot assignment for one core. Returns (idxs int16, ohpos f32)."""
    lo, hi = node_base, node_base + cfg.node_per_core
    m = (col >= lo) & (col < hi)
    lcol = (col[m] - lo).astype(np.int64)
    lrow = row[m].astype(np.int64)

    w = lcol >> 7
    pos = lcol & 127
    ck = lrow // cfg.chunk_rows
    lidx = lrow - ck * cfg.chunk_rows

    key = w * NCHUNK + ck
    order = np.argsort(key, kind="stable")
    key_s = key[order]
    pos_s = pos[order]
    lidx_s = lidx[order]

    nseg = cfg.W * NCHUNK
    counts = np.bincount(key_s, minlength=nseg)
    if counts.max(initial=0) > cfg.cap:
        raise ValueError(
            f"segment overflow: {counts.max()} > {cfg.cap}"
        )
    starts = np.zeros(nseg, np.int64)
    np.cumsum(counts[:-1], out=starts[1:])
    rank = np.arange(len(key_s)) - starts[key_s]

    w_e = key_s // NCHUNK
    r_e = key_s % NCHUNK
    tcol = r_e * cfg.region_cols + w_e * cfg.seg_tiles + (rank >> 7)
    lane = rank & 127

    srcidx = np.zeros((P, cfg.NT), np.int16)
    ohpos = np.full((P, cfg.NT), -1.0, np.float32)
    srcidx[lane, tcol] = lidx_s
    ohpos[lane, tcol] = pos_s

    # per (chunk, group) wrapped int16 index blocks: call order i = j*128+p,
    # wrapped in 16 partitions and replicated 8x (what the Q7 pairs read)
    idxs = np.zeros((P, NCHUNK * cfg.idx_cols), np.int16)
    CT = cfg.call_tiles
    ci16 = cfg.call_idxs // 16
    for r in range(NCHUNK):
        for g in range(cfg.n_groups):
            cols = r * cfg.region_cols + g * CT + np.arange(CT)
            flat = srcidx[:, cols].T.reshape(-1)
            wrapped = flat.reshape(ci16, 16).T
            c0 = r * cfg.idx_cols + g * ci16
            idxs[:, c0:c0 + ci16] = np.tile(wrapped, (8, 1))
    return idxs, ohpos


def prep_all(x, edge_index, cfg: Cfg):
    row = np.asarray(edge_index[0])
    col = np.asarray(edge_index[1])
    mdt_np = mybir.dt.np(mybir.dt.bfloat16 if cfg.bf16 else mybir.dt.float32)
    xf = np.ascontiguousarray(np.asarray(x, dtype=np.float32).astype(mdt_np))
    it = np.tile(np.arange(P, dtype=np.float32), (P, 1)).astype(mdt_np)
    in_maps = []
    for c in range(N_CORES):
        idxs, ohpos = prep_core(row, col, c * cfg.node_per_core, cfg)
        in_maps.append({"x": xf, "idxs": idxs, "ohpos": ohpos,
                        "iota128": it})
    return in_maps


class SpmdRunner:
    """PJRT SPMD runner for a prebuilt Bass module.

    Mirrors bass2jax.run_bass_via_pjrt but stages inputs with per-device
    device_put + make_array_from_single_device_arrays and reads outputs
    shard-by-shard: no host<->global-array slicing ops get compiled (this
    toolchain's penguin DataLocalityOpt rejects them for large arrays).
    """

    def __init__(self, nc: bass.Bass, n_cores: int = N_CORES):
        bass2jax.install_neuronx_cc_hook()
        self.nc = nc
        self.n_cores = n_cores
        pname = nc.partition_id_tensor.name if nc.partition_id_tensor else None
        self.partition_name = pname
        in_names, out_names, out_avals = [], [], []
        for alloc in nc.m.functions[0].allocations:
            if not isinstance(alloc, mybir.MemoryLocationSet):
                continue
            name = alloc.memorylocations[0].name
            if alloc.kind == "ExternalInput":
                if name != pname:
                    in_names.append(name)
            elif alloc.kind == "ExternalOutput":
                out_names.append(name)
                out_avals.append(
                    jax.core.ShapedArray(
                        tuple(alloc.tensor_shape), mybir.dt.np(alloc.dtype)
                    )
                )
        self.in_names = in_names
        self.out_names = out_names
        self.out_avals = out_avals
        self.devices = jax.devices()[:n_cores]
        self.mesh = Mesh(np.asarray(self.devices), ("core",))
        self.sharding = NamedSharding(self.mesh, PartitionSpec("core"))
        all_in_names = list(in_names) + list(out_names)
        if pname is not None:
            all_in_names.append(pname)

        def _body(*args):
            operands = list(args)
            if pname is not None:
                operands.append(bass2jax.partition_id_tensor())
            return tuple(
                bass2jax._bass_exec_p.bind(
                    *operands,
                    out_avals=tuple(out_avals),
                    in_names=tuple(all_in_names),
                    out_names=tuple(out_names),
                    lowering_input_output_aliases=(),
                    sim_require_finite=True,
                    sim_require_nnan=True,
                    nc=nc,
                )
            )

        n_args = len(in_names) + len(out_names)
        self.fn = jax.jit(
            shard_map(
                _body,
                mesh=self.mesh,
                in_specs=(PartitionSpec("core"),) * n_args,
                out_specs=(PartitionSpec("core"),) * len(out_names),
                check_rep=False,
            ),
            keep_unused=True,
        )

    def _global(self, per_core_arrays):
        shape = per_core_arrays[0].shape
        gshape = (self.n_cores * shape[0],) + tuple(shape[1:])
        bufs = [
            jax.device_put(a, d)
            for a, d in zip(per_core_arrays, self.devices)
        ]
        return jax.make_array_from_single_device_arrays(
            gshape, self.sharding, bufs
        )

    def stage(self, in_maps):
        args = [
            self._global([np.asarray(m[name]) for m in in_maps])
            for name in self.in_names
        ]
        args += [
            self._global(
                [np.zeros(av.shape, av.dtype) for _ in range(self.n_cores)]
            )
            for av in self.out_avals
        ]
        return args

    def run(self, args):
        outs = self.fn(*args)
        jax.block_until_ready(outs)
        return outs

    def to_numpy(self, outs):
        res = [dict() for _ in range(self.n_cores)]
        for i, name in enumerate(self.out_names):
            shards = sorted(
                outs[i].addressable_shards,
                key=lambda s: s.index[0].start or 0,
            )
            assert len(shards) == self.n_cores
            for c, s in enumerate(shards):
                res[c][name] = np.asarray(s.data)
        return res

    def __call__(self, in_maps):
        return self.to_numpy(self.run(self.stage(in_maps)))


_NC_CACHE = {}
_RUNNER_CACHE = {}


def _get_nc(cfg: Cfg) -> bass.Bass:
    nc = _NC_CACHE.get(cfg)
    if nc is None:
        nc = build(cfg)
        _NC_CACHE[cfg] = nc
    return nc


def _get_runner(cfg: Cfg) -> SpmdRunner:
    r = _RUNNER_CACHE.get(cfg)
    if r is None:
        r = SpmdRunner(_get_nc(cfg))
        _RUNNER_CACHE[cfg] = r
    return r


def _get_runner_v2(cfg: Cfg, layout) -> SpmdRunner:
    key = (cfg, layout)
    r = _RUNNER_CACHE.get(key)
    if r is None:
        nc = _NC_CACHE.get(key)
        if nc is None:
            nc = build_v2(cfg, layout)
            _NC_CACHE[key] = nc
        r = SpmdRunner(nc)
        _RUNNER_CACHE[key] = r
    return r


def _host_fallback(x, edge_index):
    out = np.zeros((x.shape[0], x.shape[1]), np.float32)
    np.add.at(
        out,
        np.asarray(edge_index[1], np.int64),
        np.asarray(x, np.float32)[np.asarray(edge_index[0], np.int64)],
    )
    return out


def kernel(x: np.ndarray, edge_index: np.ndarray) -> np.ndarray:
    x = np.asarray(x)
    edge_index = np.asarray(edge_index)
    if CFG.v2:
        try:
            layout, in_maps = prep_v2(x, edge_index, CFG)
            res = _get_runner_v2(CFG, layout)(in_maps)
        except ValueError:
            # Degenerate edge distribution (empty segment): host fallback
            # rather than returning garbage.
            return _host_fallback(x, edge_index)
        return np.concatenate(
            [res[c]["out"][: CFG.node_per_core] for c in range(N_CORES)]
        )
    try:
        in_maps = prep_all(x, edge_index, CFG)
    except ValueError:
        # Segment-capacity overflow (an edge distribution far from this
        # problem's uniform random graph): fall back to a host computation
        # rather than returning wrong results.
        return _host_fallback(x, edge_index)
    res = _get_runner(CFG)(in_maps)
    return np.concatenate(
        [res[c]["out"][: CFG.node_per_core] for c in range(N_CORES)]
    )



# revision 28
# speedup vs baseline: 2.4464x; 1.0009x over previous
"""Trainium2 Bass kernel for GNN message passing (gather + segment_sum).

reference:
    row, col = edge_index
    out = segment_sum(x[row], col, num_segments=x.shape[0])    # [100000, 128]

Architecture (destination-sharded one-hot-matmul scatter-add, no collectives):
- Host: shard destination nodes contiguously across 8 cores (12500/core).
  Per core, dests are grouped into 98 windows of 128 nodes. Each window has 4
  source-chunk segments (x split into 4 row-quarters of 25000 so dma_gather's
  int16 indices can address them) of 2 tiles (256 edge slots) each -> 8 tiles
  per window, 784 tiles per core. Segment overflow is checked (max observed
  250/256 for this problem's uniform graph); on overflow a host fallback
  computes the answer instead of returning garbage.
- Device (identical SPMD program on 8 cores; only the data differs):
  * Messages, one-hots, and matmuls run in bf16 (CFG.bf16): host casts x to
    bf16 (256B rows), halving HBM gather payload; PE runs 1 cycle/row vs 4
    for fp32, and the one-hot is_equal gets the 4x DVE mode. PSUM
    accumulates fp32, so only the input rounding (~2^-9) is lost; measured
    rel err 1.9e-03 vs the 2e-02 gate.
  * dma_gather (ANT extended SWDGE op) pulls x-rows for each edge slot,
    one call per (window-group, source-chunk), spread over 4 SWDGE queues so
    all four Q7 core-pairs generate descriptors in parallel. gbufs=4 gather
    buffers per chunk keep the Pool/DMA pipeline 3 groups ahead of compute.
  * Per edge tile: one-hot OH[e,d] = (ohpos[e] == iota_d) via VectorE
    tensor_scalar is_equal (padding slots carry ohpos=-1 -> zero row), then
    TensorE matmul psum[d,f] += OH.T @ msg accumulated over the window's 8
    tiles.
  * Per window: PSUM -> a persistent SBUF output buffer via ScalarE copy
    (CFG.bigout); one strided DMA per 7-window group flushes to HBM, so no
    per-window DMA ever gates PSUM recycling behind the gather's descriptor
    queue on the shared DMA engines.
- Host: concatenate the 8 per-core output slices.

An alternative exact-count layout (prep_v2/build_v2, CFG.v2) packs edges
contiguously per (group, chunk) with per-core valid counts fed to the
gather via Pool reg_load, cutting descriptors 100352 -> 78125/core; tiles
may span two windows, handled by a 256-wide one-hot over iota256 with
compare value pos + 128*k. It validates (rel err 1.9e-03) but measured no
faster end-to-end on this toolchain, so the static v1 layout ships.
"""

from dataclasses import dataclass

import numpy as np

import jax
from jax.experimental.shard_map import shard_map
from jax.sharding import Mesh, NamedSharding, PartitionSpec

import concourse.bass as bass
import concourse.mybir as mybir
import concourse.tile as tile
from concourse import bass2jax
from concourse.vector_clock import ScopedClock

# ---------------------------------------------------------------------------
# Toolchain workarounds for this walrus build:
# 1) The ISA here allows at most ONE sync-wait command per instruction
#    ("Too many sync wait commands" at codegen otherwise). TileContext's tail
#    drain carries one wait per live semaphore lane, and the scheduler can
#    attach several waits to body instructions too, so every surplus wait is
#    moved onto its own same-engine NOP placed directly before the original
#    instruction (the sequencer executes them in order — semantics identical).
# 2) Extended/pseudo Pool instructions (load_library, dma_gather) need
#    codegen_inst_isa_subclasses before walrus, else "ISA wrong length".
# ---------------------------------------------------------------------------


def _drain_and_barrier_split(self, tick_clock, wait_clock):
    nc = self.nc
    drain_inst = nc.sync.drain()
    wait_clock.add_sem_waits(
        drain_inst.ins, ScopedClock({None: tick_clock.global_clock})
    )
    si = drain_inst.ins.sync_info
    if si is not None and len(si.on_wait) > 0:
        waits = list(si.on_wait)
        si.on_wait = []
        for w in waits:
            nop = nc.sync.nop(nofuse=True)
            nop.ins.sync_info = mybir.SyncInfo(on_wait=[w], on_update=[])
    nc.all_engine_barrier()
    assert self.sems is not None
    popped = nc._tile_sem_poison_stack.pop()
    assert popped is self._sem_poison
    nc.clear_and_free_semaphores(list(self.sems.allocated().values()))
    nc.all_engine_barrier()


tile.TileContext._drain_and_barrier = _drain_and_barrier_split


def split_multi_waits(nc: "bass.Bass", max_waits: int = 1) -> None:
    k = 0
    for fn in nc.m.functions:
        for bb in fn.blocks:
            il = list(bb.instructions)
            out = []
            changed = False
            for inst in il:
                si = inst.sync_info
                if si is not None and len(si.on_wait) > max_waits:
                    waits = list(si.on_wait)
                    si.on_wait = waits[:max_waits]
                    for w in waits[max_waits:]:
                        nop = mybir.InstNoOp(
                            name=f"I-wsplit-{k}", ins=[], outs=[]
                        )
                        k += 1
                        nop.engine = inst.engine
                        nop.sync_info = mybir.SyncInfo(
                            on_wait=[w], on_update=[]
                        )
                        nc.register_instruction(nop, overwrite=True)
                        out.append(nop)
                        changed = True
                out.append(inst)
            if changed:
                bb.instructions = out


# ---------------------------------------------------------------------------
# Kernel
# ---------------------------------------------------------------------------

D = 128
P = 128
N_CORES = 8
NCHUNK = 4  # x row-quarters (int16 dma_gather index reach)


@dataclass(frozen=True)
class Cfg:
    n_nodes: int
    node_per_core: int
    seg_tiles: int  # tiles per (window, chunk) segment
    group_w: int  # windows per gather group
    gbufs: int = 2
    bf16: bool = False  # gather/one-hot/matmul in bf16 (PSUM stays f32)
    scratch: int = 65536  # SWDGE descriptor-ring carveout bytes
    bigout: bool = False  # accumulate output in SBUF, flush per group
    psum_bufs: int = 4
    v2: bool = False  # exact-count packed layout (build_v2/prep_v2)
    phases: int = 0  # >0: serialize gather/compute in this many phases
    indirect: bool = False  # gather via HWDGE indirect_dma_start (no Q7)

    @property
    def chunk_rows(self) -> int:
        return self.n_nodes // NCHUNK

    @property
    def W(self) -> int:
        return -(-self.node_per_core // P)

    @property
    def T(self) -> int:
        return NCHUNK * self.seg_tiles

    @property
    def NT(self) -> int:
        return self.W * self.T

    @property
    def region_cols(self) -> int:
        return self.W * self.seg_tiles

    @property
    def n_groups(self) -> int:
        assert self.W % self.group_w == 0
        return self.W // self.group_w

    @property
    def call_tiles(self) -> int:
        return self.group_w * self.seg_tiles

    @property
    def call_idxs(self) -> int:
        return self.call_tiles * P

    @property
    def idx_cols(self) -> int:
        return self.n_groups * (self.call_idxs // 16)

    @property
    def out_rows(self) -> int:
        return self.W * P

    @property
    def cap(self) -> int:
        return self.seg_tiles * P


CFG = Cfg(n_nodes=100000, node_per_core=12500, seg_tiles=2, group_w=7,
          gbufs=4, bf16=True, bigout=True, psum_bufs=8)


def build(cfg: Cfg, reps: int = 1) -> bass.Bass:
    from concourse.library_config import mlp
    from concourse.library_overlay import lower_extended_insts

    assert cfg.chunk_rows <= 32767
    mdt = mybir.dt.bfloat16 if cfg.bf16 else mybir.dt.float32
    nc = bass.Bass(num_swdge_queues=NCHUNK,
                   dynamic_dma_scratch_size=cfg.scratch)
    x = nc.declare_dram_parameter("x", [cfg.n_nodes, D], mdt,
                                  isOutput=False)
    if cfg.indirect:
        idxs = nc.declare_dram_parameter(
            "idxs", [P, cfg.NT], mybir.dt.int32, isOutput=False)
    else:
        idxs = nc.declare_dram_parameter(
            "idxs", [P, NCHUNK * cfg.idx_cols], mybir.dt.int16,
            isOutput=False)
    ohpos = nc.declare_dram_parameter("ohpos", [P, cfg.NT], mybir.dt.float32,
                                      isOutput=False)
    iota = nc.declare_dram_parameter("iota128", [P, P], mdt,
                                     isOutput=False)
    out = nc.declare_dram_parameter("out", [cfg.out_rows, D],
                                    mybir.dt.float32, isOutput=True)

    CT = cfg.call_tiles
    nc.gpsimd.load_library(mlp)
    with tile.TileContext(nc) as tc:
        with (
            tc.tile_pool(name="tabs", bufs=1) as tabs,
            tc.tile_pool(name="gbuf", bufs=cfg.gbufs) as gbuf,
            tc.tile_pool(name="ohb", bufs=4) as ohb,
            tc.tile_pool(name="psumb", bufs=cfg.psum_bufs,
                         space="PSUM") as psumb,
            tc.tile_pool(name="outb", bufs=4) as outb,
        ):
            idxs_sb = tabs.tile(
                [P, cfg.NT] if cfg.indirect else [P, NCHUNK * cfg.idx_cols],
                mybir.dt.int32 if cfg.indirect else mybir.dt.int16)
            ohpos_sb = tabs.tile([P, cfg.NT], mybir.dt.float32)
            iota_sb = tabs.tile([P, P], mdt)
            nc.sync.dma_start(out=idxs_sb[:], in_=idxs[:])
            nc.sync.dma_start(out=ohpos_sb[:], in_=ohpos[:])
            nc.sync.dma_start(out=iota_sb[:], in_=iota[:])
            outsb = (tabs.tile([P, cfg.W * D], mybir.dt.float32,
                                name="outsb", tag="outsb")
                     if cfg.bigout else None)

            # one shared register for the gather count — a fresh to_reg per
            # call exhausts the Pool register file at this call count
            nidx_reg = nc.gpsimd.to_reg(cfg.call_idxs)

            for g in range(cfg.n_groups * reps):
                g = g % cfg.n_groups
                chts = []
                for r in range(NCHUNK):
                    ch = gbuf.tile([P, CT * D], mdt,
                                   tag=f"ch{r}")
                    if cfg.indirect:
                        i0 = r * cfg.region_cols + g * CT
                        nc.gpsimd.indirect_dma_start(
                            out=ch[:].rearrange("p (c d) -> p c d", c=CT),
                            out_offset=None,
                            in_=x[:, :],
                            in_offset=bass.IndirectOffsetOnAxis(
                                ap=idxs_sb[:, i0:i0 + CT],
                                axis=0,
                            ),
                        )
                        chts.append(ch)
                        continue
                    c0 = r * cfg.idx_cols + g * (cfg.call_idxs // 16)
                    nc.gpsimd.dma_gather(
                        ch[:].rearrange("p (c d) -> p c d", c=CT),
                        x[r * cfg.chunk_rows:(r + 1) * cfg.chunk_rows, :],
                        idxs_sb[:, c0:c0 + cfg.call_idxs // 16],
                        cfg.call_idxs,
                        nidx_reg,
                        D,
                        queue_num=r,
                        # single_packet coalesces the whole descriptor stream
                        # into one SDMA packet; beyond ~1K descriptors that
                        # wedges the DMA engine (HW hang). Multi-packet is
                        # required at this call size.
                        single_packet=False,
                    )
                    chts.append(ch)
                for wl in range(cfg.group_w):
                    w = g * cfg.group_w + wl
                    ps = psumb.tile([P, D], mybir.dt.float32, tag="ps")
                    k = 0
                    last = cfg.T - 1
                    for r in range(NCHUNK):
                        for i in range(cfg.seg_tiles):
                            tcol = (r * cfg.region_cols
                                    + cfg.seg_tiles * w + i)
                            lcol = cfg.seg_tiles * wl + i
                            oh = ohb.tile([P, P], mdt, tag="oh")
                            nc.vector.tensor_scalar(
                                out=oh[:],
                                in0=iota_sb[:],
                                scalar1=ohpos_sb[:, tcol:tcol + 1],
                                scalar2=None,
                                op0=mybir.AluOpType.is_equal,
                            )
                            nc.tensor.matmul(
                                ps[:],
                                lhsT=oh[:],
                                rhs=chts[r][:, lcol * D:(lcol + 1) * D],
                                start=(k == 0),
                                stop=(k == last),
                            )
                            k += 1
                    if cfg.bigout:
                        nc.scalar.copy(out=outsb[:, w * D:(w + 1) * D],
                                       in_=ps[:])
                        if wl == cfg.group_w - 1:
                            w0 = g * cfg.group_w
                            nc.sync.dma_start(
                                out=out[w0 * P:(w + 1) * P, :]
                                .rearrange("(w p) d -> p w d",
                                           w=cfg.group_w),
                                in_=outsb[:, w0 * D:(w + 1) * D]
                                .rearrange("p (w d) -> p w d",
                                           w=cfg.group_w),
                            )
                    else:
                        ob = outb.tile([P, D], mybir.dt.float32, tag="ob")
                        nc.scalar.copy(out=ob[:], in_=ps[:])
                        nc.sync.dma_start(
                            out=out[w * P:(w + 1) * P, :], in_=ob[:]
                        )
    split_multi_waits(nc)
    lower_extended_insts(nc)
    return nc


# ---------------------------------------------------------------------------
# v2 layout: exact-count packed gather (no intra-segment padding)
#
# Edges are packed contiguously per (group, chunk) sorted by window; gather
# calls carry per-core valid counts in a Pool register (reg_load), so padding
# slots generate no DMA descriptors.  A tile may span up to two consecutive
# windows; the per-tile window list (union across all 8 cores, so one shared
# SPMD program works) drives one wide one-hot per <=2 windows: OH built over
# iota256 with compare value pos + 128*k, then one matmul per 128-col slice.
# ---------------------------------------------------------------------------


def prep_v2(x, edge_index, cfg: Cfg):
    row = np.asarray(edge_index[0]).astype(np.int64)
    col = np.asarray(edge_index[1]).astype(np.int64)
    mdt_np = mybir.dt.np(mybir.dt.bfloat16 if cfg.bf16 else mybir.dt.float32)
    xf = np.ascontiguousarray(np.asarray(x, dtype=np.float32).astype(mdt_np))
    NG, GW = cfg.n_groups, cfg.group_w

    cores = []
    wcounts = np.zeros((N_CORES, NG, NCHUNK, GW), np.int64)
    for c in range(N_CORES):
        lo = c * cfg.node_per_core
        m = (col >= lo) & (col < lo + cfg.node_per_core)
        lcol = col[m] - lo
        lrow = row[m]
        w = lcol >> 7
        pos = lcol & 127
        ck = lrow // cfg.chunk_rows
        lidx = lrow - ck * cfg.chunk_rows
        g = w // GW
        wl = w % GW
        order = np.lexsort((wl, ck, g))
        cores.append((g[order], ck[order], wl[order], pos[order],
                      lidx[order]))
        np.add.at(wcounts[c], (g[order], ck[order], wl[order]), 1)

    C = wcounts.sum(axis=3)  # [core, g, ck]
    if C.min() < 1:
        raise ValueError("v2 layout needs >=1 edge per (core, group, chunk)")
    T = -(-C.max(axis=0) // P)  # [g, ck] static tile counts

    winlists = []  # [g][ck] -> tuple per tile of sorted wl tuple
    for g in range(NG):
        row_l = []
        for ck in range(NCHUNK):
            tiles = [set() for _ in range(int(T[g, ck]))]
            for c in range(N_CORES):
                posn = 0
                for wl in range(GW):
                    n = int(wcounts[c, g, ck, wl])
                    if n == 0:
                        continue
                    for t in range(posn // P, (posn + n - 1) // P + 1):
                        tiles[t].add(wl)
                    posn += n
            row_l.append(tuple(tuple(sorted(s)) for s in tiles))
        winlists.append(tuple(row_l))
    Cmin = C.min(axis=0)  # [g, ck] min valid count across cores
    layout = (tuple(tuple(int(v) for v in T[g]) for g in range(NG)),
              tuple(winlists),
              tuple(tuple(int(v) for v in Cmin[g]) for g in range(NG)))

    # static oh-instruction schedule: per (g, ck, tile) split winlist into
    # chunks of <=2; each chunk is one ohpos column
    n_oh = 0
    for g in range(NG):
        for ck in range(NCHUNK):
            for wl_list in winlists[g][ck]:
                n_oh += max(1, -(-len(wl_list) // 2))
    Tmax = int(T.max())
    ci16_total = int(T.sum()) * 8

    in_maps = []
    for c in range(N_CORES):
        gs, cks, wls, poss, lidxs = cores[c]
        idxs = np.zeros((16, ci16_total), np.int16)
        ohpos = np.full((P, n_oh), -1.0, np.float32)
        counts = np.zeros((P, NG * NCHUNK), np.int32)
        base = 0
        ohcol = 0
        e0 = 0
        # edges are sorted by (g, ck, wl); walk segments in order
        for g in range(NG):
            for ck in range(NCHUNK):
                n = int(C[c, g, ck])
                tcount = int(T[g, ck])
                nslots = tcount * P
                sl = slice(e0, e0 + n)
                stream = np.full(nslots, -1, np.int16)
                stream[:n] = lidxs[sl]
                counts[:, g * NCHUNK + ck] = n
                idxs[:, base:base + tcount * 8] = (
                    stream.reshape(tcount * 8, 16).T
                )
                base += tcount * 8
                # oh columns for this segment
                wl_seg = wls[sl]
                pos_seg = poss[sl]
                for t, wl_list in enumerate(winlists[g][ck]):
                    s0, s1 = t * P, min((t + 1) * P, nslots)
                    nvals = max(0, min(s1, n) - s0)
                    lanes = np.arange(s0, s0 + nvals) - s0
                    for pair_i in range(max(1, -(-len(wl_list) // 2))):
                        pair = wl_list[2 * pair_i:2 * pair_i + 2]
                        if nvals > 0:
                            wl_t = wl_seg[s0:s0 + nvals]
                            pos_t = pos_seg[s0:s0 + nvals]
                            for k, wl in enumerate(pair):
                                mk = wl_t == wl
                                ohpos[lanes[mk], ohcol] = (
                                    pos_t[mk] + 128 * k
                                )
                        ohcol += 1
                e0 += n
        assert ohcol == n_oh and base == ci16_total
        it = np.tile(np.arange(256, dtype=np.float32), (P, 1)).astype(mdt_np)
        in_maps.append({"x": xf, "idxs": np.tile(idxs, (8, 1)),
                        "ohpos": ohpos, "iota256": it,
                        "counts": counts})
    return layout, in_maps


def build_v2(cfg: Cfg, layout, reps: int = 1) -> bass.Bass:
    from concourse.library_config import mlp
    from concourse.library_overlay import lower_extended_insts

    T, winlists, Cmin = layout
    NG, GW = cfg.n_groups, cfg.group_w
    Tmax = max(max(r) for r in T)
    ci16_total = sum(sum(r) for r in T) * 8
    n_oh = sum(max(1, -(-len(wl) // 2))
               for g in range(NG) for ck in range(NCHUNK)
               for wl in winlists[g][ck])
    mdt = mybir.dt.bfloat16 if cfg.bf16 else mybir.dt.float32

    nc = bass.Bass(num_swdge_queues=NCHUNK,
                   dynamic_dma_scratch_size=cfg.scratch)
    x = nc.declare_dram_parameter("x", [cfg.n_nodes, D], mdt, isOutput=False)
    idxs = nc.declare_dram_parameter("idxs", [P, ci16_total], mybir.dt.int16,
                                     isOutput=False)
    ohpos = nc.declare_dram_parameter("ohpos", [P, n_oh], mybir.dt.float32,
                                      isOutput=False)
    iota = nc.declare_dram_parameter("iota256", [P, 2 * P], mdt,
                                     isOutput=False)
    counts = nc.declare_dram_parameter("counts", [P, NG * NCHUNK],
                                       mybir.dt.int32, isOutput=False)
    out = nc.declare_dram_parameter("out", [cfg.out_rows, D],
                                    mybir.dt.float32, isOutput=True)

    nc.gpsimd.load_library(mlp)
    with tile.TileContext(nc) as tc:
        with (
            tc.tile_pool(name="tabs", bufs=1) as tabs,
            tc.tile_pool(name="gbuf", bufs=cfg.gbufs) as gbuf,
            tc.tile_pool(name="ohb", bufs=6) as ohb,
            tc.tile_pool(name="psumb", bufs=1, space="PSUM") as psumb,
        ):
            idxs_sb = tabs.tile([P, ci16_total], mybir.dt.int16)
            ohpos_sb = tabs.tile([P, n_oh], mybir.dt.float32)
            iota_sb = tabs.tile([P, 2 * P], mdt)
            counts_sb = tabs.tile([P, NG * NCHUNK], mybir.dt.int32)
            outsb = tabs.tile([P, cfg.W * D], mybir.dt.float32,
                              name="outsb", tag="outsb")
            nc.sync.dma_start(out=idxs_sb[:], in_=idxs[:])
            nc.sync.dma_start(out=ohpos_sb[:], in_=ohpos[:])
            nc.sync.dma_start(out=iota_sb[:], in_=iota[:])
            nc.sync.dma_start(out=counts_sb[:], in_=counts[:])

            cnt_reg = nc.gpsimd.to_reg(0)

            # static idx base offsets per (g, ck)
            bases = {}
            b = 0
            for g in range(NG):
                for ck in range(NCHUNK):
                    bases[(g, ck)] = b
                    b += T[g][ck] * 8

            # per-group matmul totals per window for start/stop flags
            mm_per_wl = []
            for g in range(NG):
                cnt = {wl: 0 for wl in range(GW)}
                for ck in range(NCHUNK):
                    for wl_list in winlists[g][ck]:
                        for wl in wl_list:
                            cnt[wl] += 1
                mm_per_wl.append(cnt)

            if cfg.phases:
                ppg = -(-NG // cfg.phases)  # groups per phase
                phase_groups = [list(range(p0, min(p0 + ppg, NG)))
                                for p0 in range(0, NG, ppg)]
            else:
                phase_groups = [[g] for g in range(NG)]
            all_chts = {}
            for pseq in range(len(phase_groups) * reps):
                glist = phase_groups[pseq % len(phase_groups)]
                for g in glist:
                    all_chts[g] = {}
                    chts = all_chts[g]
                    for ck in range(NCHUNK):
                        tcount = T[g][ck]
                        ch = gbuf.tile([P, Tmax * D], mdt, tag=f"ch{ck}")
                        c0 = bases[(g, ck)]
                        # zero the tiles past every core's valid count: the
                        # gather skips trailing -1 slots, and 0 * one-hot-0
                        # keeps them out of the sums (NaN-safe on fresh SBUF)
                        t0 = Cmin[g][ck] // P
                        nc.vector.memset(ch[:, t0 * D:tcount * D], 0.0)
                        nc.gpsimd.reg_load(
                            cnt_reg,
                            counts_sb[0:1,
                                      g * NCHUNK + ck:g * NCHUNK + ck + 1],
                        )
                        nc.gpsimd.dma_gather(
                            ch[:, :tcount * D].rearrange(
                                "p (c d) -> p c d", c=tcount),
                            x[ck * cfg.chunk_rows:
                              (ck + 1) * cfg.chunk_rows, :],
                            idxs_sb[:, c0:c0 + tcount * 8],
                            tcount * P,
                            cnt_reg,
                            D,
                            queue_num=ck,
                            single_packet=False,
                        )
                        chts[ck] = ch
                for g in (reversed(glist) if cfg.phases else glist):
                    chts = all_chts[g]
                    pss = {wl: psumb.tile([P, D], mybir.dt.float32,
                                          name=f"ps{wl}", tag=f"ps{wl}")
                           for wl in range(GW)}
                    seen = {wl: 0 for wl in range(GW)}
                    ohcol = sum(
                        max(1, -(-len(wl_list) // 2))
                        for gg in range(g)
                        for ck in range(NCHUNK)
                        for wl_list in winlists[gg][ck]
                    )
                    for ck in range(NCHUNK):
                        for t, wl_list in enumerate(winlists[g][ck]):
                            npair = max(1, -(-len(wl_list) // 2))
                            for pair_i in range(npair):
                                pair = wl_list[2 * pair_i:2 * pair_i + 2]
                                width = P * max(1, len(pair))
                                oh = ohb.tile([P, width], mdt,
                                              tag=f"oh{len(pair)}")
                                nc.vector.tensor_scalar(
                                    out=oh[:],
                                    in0=iota_sb[:, :width],
                                    scalar1=ohpos_sb[:, ohcol:ohcol + 1],
                                    scalar2=None,
                                    op0=mybir.AluOpType.is_equal,
                                )
                                for k, wl in enumerate(pair):
                                    seen[wl] += 1
                                    nc.tensor.matmul(
                                        pss[wl][:],
                                        lhsT=oh[:, k * P:(k + 1) * P],
                                        rhs=chts[ck][:, t * D:(t + 1) * D],
                                        start=(seen[wl] == 1),
                                        stop=(seen[wl] == mm_per_wl[g][wl]),
                                    )
                                ohcol += 1
                    for wl in range(GW):
                        w = g * GW + wl
                        nc.scalar.copy(out=outsb[:, w * D:(w + 1) * D],
                                       in_=pss[wl][:])
                    w0 = g * GW
                    nc.sync.dma_start(
                        out=out[w0 * P:(w0 + GW) * P, :]
                        .rearrange("(w p) d -> p w d", w=GW),
                        in_=outsb[:, w0 * D:(w0 + GW) * D]
                        .rearrange("p (w d) -> p w d", w=GW),
                    )
    split_multi_waits(nc)
    lower_extended_insts(nc)
    return nc


def prep_core(row, col, node_base, cfg: Cfg):
    """Slot assignment for one core. Returns (idxs int16, ohpos f32)."""
    lo, hi = node_base, node_base + cfg.node_per_core
    m = (col >= lo) & (col < hi)
    lcol = (col[m] - lo).astype(np.int64)
    lrow = row[m].astype(np.int64)

    w = lcol >> 7
    pos = lcol & 127
    ck = lrow // cfg.chunk_rows
    lidx = lrow - ck * cfg.chunk_rows

    key = w * NCHUNK + ck
    order = np.argsort(key, kind="stable")
    key_s = key[order]
    pos_s = pos[order]
    lidx_s = lidx[order]

    nseg = cfg.W * NCHUNK
    counts = np.bincount(key_s, minlength=nseg)
    if counts.max(initial=0) > cfg.cap:
        raise ValueError(
            f"segment overflow: {counts.max()} > {cfg.cap}"
        )
    starts = np.zeros(nseg, np.int64)
    np.cumsum(counts[:-1], out=starts[1:])
    rank = np.arange(len(key_s)) - starts[key_s]

    w_e = key_s // NCHUNK
    r_e = key_s % NCHUNK
    tcol = r_e * cfg.region_cols + w_e * cfg.seg_tiles + (rank >> 7)
    lane = rank & 127

    srcidx = np.zeros((P, cfg.NT), np.int16)
    ohpos = np.full((P, cfg.NT), -1.0, np.float32)
    srcidx[lane, tcol] = lidx_s
    ohpos[lane, tcol] = pos_s

    if cfg.indirect:
        # absolute int32 row index per slot, addressed [lane, tcol]
        # directly by the HWDGE indirect gather (no 16-wrap, no chunks)
        idxs = np.zeros((P, cfg.NT), np.int32)
        idxs[lane, tcol] = (r_e * cfg.chunk_rows + lidx_s).astype(np.int32)
        return idxs, ohpos

    # per (chunk, group) wrapped int16 index blocks: call order i = j*128+p,
    # wrapped in 16 partitions and replicated 8x (what the Q7 pairs read)
    idxs = np.zeros((P, NCHUNK * cfg.idx_cols), np.int16)
    CT = cfg.call_tiles
    ci16 = cfg.call_idxs // 16
    for r in range(NCHUNK):
        for g in range(cfg.n_groups):
            cols = r * cfg.region_cols + g * CT + np.arange(CT)
            flat = srcidx[:, cols].T.reshape(-1)
            wrapped = flat.reshape(ci16, 16).T
            c0 = r * cfg.idx_cols + g * ci16
            idxs[:, c0:c0 + ci16] = np.tile(wrapped, (8, 1))
    return idxs, ohpos


def prep_all(x, edge_index, cfg: Cfg):
    row = np.asarray(edge_index[0])
    col = np.asarray(edge_index[1])
    mdt_np = mybir.dt.np(mybir.dt.bfloat16 if cfg.bf16 else mybir.dt.float32)
    xf = np.ascontiguousarray(np.asarray(x, dtype=np.float32).astype(mdt_np))
    it = np.tile(np.arange(P, dtype=np.float32), (P, 1)).astype(mdt_np)
    in_maps = []
    for c in range(N_CORES):
        idxs, ohpos = prep_core(row, col, c * cfg.node_per_core, cfg)
        in_maps.append({"x": xf, "idxs": idxs, "ohpos": ohpos,
                        "iota128": it})
    return in_maps


class SpmdRunner:
    """PJRT SPMD runner for a prebuilt Bass module.

    Mirrors bass2jax.run_bass_via_pjrt but stages inputs with per-device
    device_put + make_array_from_single_device_arrays and reads outputs
    shard-by-shard: no host<->global-array slicing ops get compiled (this
    toolchain's penguin DataLocalityOpt rejects them for large arrays).
    """

    def __init__(self, nc: bass.Bass, n_cores: int = N_CORES):
        bass2jax.install_neuronx_cc_hook()
        self.nc = nc
        self.n_cores = n_cores
        pname = nc.partition_id_tensor.name if nc.partition_id_tensor else None
        self.partition_name = pname
        in_names, out_names, out_avals = [], [], []
        for alloc in nc.m.functions[0].allocations:
            if not isinstance(alloc, mybir.MemoryLocationSet):
                continue
            name = alloc.memorylocations[0].name
            if alloc.kind == "ExternalInput":
                if name != pname:
                    in_names.append(name)
            elif alloc.kind == "ExternalOutput":
                out_names.append(name)
                out_avals.append(
                    jax.core.ShapedArray(
                        tuple(alloc.tensor_shape), mybir.dt.np(alloc.dtype)
                    )
                )
        self.in_names = in_names
        self.out_names = out_names
        self.out_avals = out_avals
        self.devices = jax.devices()[:n_cores]
        self.mesh = Mesh(np.asarray(self.devices), ("core",))
        self.sharding = NamedSharding(self.mesh, PartitionSpec("core"))
        all_in_names = list(in_names) + list(out_names)
        if pname is not None:
            all_in_names.append(pname)

        def _body(*args):
            operands = list(args)
            if pname is not None:
                operands.append(bass2jax.partition_id_tensor())
            return tuple(
                bass2jax._bass_exec_p.bind(
                    *operands,
                    out_avals=tuple(out_avals),
                    in_names=tuple(all_in_names),
                    out_names=tuple(out_names),
                    lowering_input_output_aliases=(),
                    sim_require_finite=True,
                    sim_require_nnan=True,
                    nc=nc,
                )
            )

        n_args = len(in_names) + len(out_names)
        self.fn = jax.jit(
            shard_map(
                _body,
                mesh=self.mesh,
                in_specs=(PartitionSpec("core"),) * n_args,
                out_specs=(PartitionSpec("core"),) * len(out_names),
                check_rep=False,
            ),
            keep_unused=True,
        )

    def _global(self, per_core_arrays):
        shape = per_core_arrays[0].shape
        gshape = (self.n_cores * shape[0],) + tuple(shape[1:])
        bufs = [
            jax.device_put(a, d)
            for a, d in zip(per_core_arrays, self.devices)
        ]
        return jax.make_array_from_single_device_arrays(
            gshape, self.sharding, bufs
        )

    def stage(self, in_maps):
        args = [
            self._global([np.asarray(m[name]) for m in in_maps])
            for name in self.in_names
        ]
        args += [
            self._global(
                [np.zeros(av.shape, av.dtype) for _ in range(self.n_cores)]
            )
            for av in self.out_avals
        ]
        return args

    def run(self, args):
        outs = self.fn(*args)
        jax.block_until_ready(outs)
        return outs

    def to_numpy(self, outs):
        res = [dict() for _ in range(self.n_cores)]
        for i, name in enumerate(self.out_names):
            shards = sorted(
                outs[i].addressable_shards,
                key=lambda s: s.index[0].start or 0,
            )
            assert len(shards) == self.n_cores
            for c, s in enumerate(shards):
                res[c][name] = np.asarray(s.data)
        return res

    def __call__(self, in_maps):
        return self.to_numpy(self.run(self.stage(in_maps)))


_NC_CACHE = {}
_RUNNER_CACHE = {}


def _get_nc(cfg: Cfg) -> bass.Bass:
    nc = _NC_CACHE.get(cfg)
    if nc is None:
        nc = build(cfg)
        _NC_CACHE[cfg] = nc
    return nc


def _get_runner(cfg: Cfg) -> SpmdRunner:
    r = _RUNNER_CACHE.get(cfg)
    if r is None:
        r = SpmdRunner(_get_nc(cfg))
        _RUNNER_CACHE[cfg] = r
    return r


def _get_runner_v2(cfg: Cfg, layout) -> SpmdRunner:
    key = (cfg, layout)
    r = _RUNNER_CACHE.get(key)
    if r is None:
        nc = _NC_CACHE.get(key)
        if nc is None:
            nc = build_v2(cfg, layout)
            _NC_CACHE[key] = nc
        r = SpmdRunner(nc)
        _RUNNER_CACHE[key] = r
    return r


def _host_fallback(x, edge_index):
    out = np.zeros((x.shape[0], x.shape[1]), np.float32)
    np.add.at(
        out,
        np.asarray(edge_index[1], np.int64),
        np.asarray(x, np.float32)[np.asarray(edge_index[0], np.int64)],
    )
    return out


def kernel(x: np.ndarray, edge_index: np.ndarray) -> np.ndarray:
    x = np.asarray(x)
    edge_index = np.asarray(edge_index)
    if CFG.v2:
        try:
            layout, in_maps = prep_v2(x, edge_index, CFG)
            res = _get_runner_v2(CFG, layout)(in_maps)
        except ValueError:
            # Degenerate edge distribution (empty segment): host fallback
            # rather than returning garbage.
            return _host_fallback(x, edge_index)
        return np.concatenate(
            [res[c]["out"][: CFG.node_per_core] for c in range(N_CORES)]
        )
    try:
        in_maps = prep_all(x, edge_index, CFG)
    except ValueError:
        # Segment-capacity overflow (an edge distribution far from this
        # problem's uniform random graph): fall back to a host computation
        # rather than returning wrong results.
        return _host_fallback(x, edge_index)
    res = _get_runner(CFG)(in_maps)
    return np.concatenate(
        [res[c]["out"][: CFG.node_per_core] for c in range(N_CORES)]
    )



# revision 29
# speedup vs baseline: 9.0641x; 3.7050x over previous
"""Trainium2 Bass kernel for GNN message passing (gather + segment_sum).

reference:
    row, col = edge_index
    out = segment_sum(x[row], col, num_segments=x.shape[0])    # [100000, 128]

Architecture (destination-sharded one-hot-matmul scatter-add, no collectives):
- Host: shard destination nodes contiguously across 8 cores (12500/core).
  Per core, dests are grouped into 98 windows of 128 nodes. Each window has 4
  source-chunk segments (x split into 4 row-quarters of 25000 so dma_gather's
  int16 indices can address them) of 2 tiles (256 edge slots) each -> 8 tiles
  per window, 784 tiles per core. Segment overflow is checked (max observed
  250/256 for this problem's uniform graph); on overflow a host fallback
  computes the answer instead of returning garbage.
- Device (identical SPMD program on 8 cores; only the data differs):
  * Messages, one-hots, and matmuls run in bf16 (CFG.bf16): host casts x to
    bf16 (256B rows), halving HBM gather payload; PE runs 1 cycle/row vs 4
    for fp32, and the one-hot is_equal gets the 4x DVE mode. PSUM
    accumulates fp32, so only the input rounding (~2^-9) is lost; measured
    rel err 1.9e-03 vs the 2e-02 gate.
  * dma_gather (ANT extended SWDGE op) pulls x-rows for each edge slot,
    one call per (window-group, source-chunk), spread over 4 SWDGE queues so
    all four Q7 core-pairs generate descriptors in parallel. gbufs=4 gather
    buffers per chunk keep the Pool/DMA pipeline 3 groups ahead of compute.
  * Per edge tile: one-hot OH[e,d] = (ohpos[e] == iota_d) via VectorE
    tensor_scalar is_equal (padding slots carry ohpos=-1 -> zero row), then
    TensorE matmul psum[d,f] += OH.T @ msg accumulated over the window's 8
    tiles.
  * Per window: PSUM -> a persistent SBUF output buffer via ScalarE copy
    (CFG.bigout); one strided DMA per 7-window group flushes to HBM, so no
    per-window DMA ever gates PSUM recycling behind the gather's descriptor
    queue on the shared DMA engines.
- Host: concatenate the 8 per-core output slices.

An alternative exact-count layout (prep_v2/build_v2, CFG.v2) packs edges
contiguously per (group, chunk) with per-core valid counts fed to the
gather via Pool reg_load, cutting descriptors 100352 -> 78125/core; tiles
may span two windows, handled by a 256-wide one-hot over iota256 with
compare value pos + 128*k. It validates (rel err 1.9e-03) but measured no
faster end-to-end on this toolchain, so the static v1 layout ships.
"""

from dataclasses import dataclass

import numpy as np

import jax
from jax.experimental.shard_map import shard_map
from jax.sharding import Mesh, NamedSharding, PartitionSpec

import concourse.bass as bass
import concourse.mybir as mybir
import concourse.tile as tile
from concourse import bass2jax
from concourse.vector_clock import ScopedClock

# ---------------------------------------------------------------------------
# Toolchain workarounds for this walrus build:
# 1) The ISA here allows at most ONE sync-wait command per instruction
#    ("Too many sync wait commands" at codegen otherwise). TileContext's tail
#    drain carries one wait per live semaphore lane, and the scheduler can
#    attach several waits to body instructions too, so every surplus wait is
#    moved onto its own same-engine NOP placed directly before the original
#    instruction (the sequencer executes them in order — semantics identical).
# 2) Extended/pseudo Pool instructions (load_library, dma_gather) need
#    codegen_inst_isa_subclasses before walrus, else "ISA wrong length".
# ---------------------------------------------------------------------------


def _drain_and_barrier_split(self, tick_clock, wait_clock):
    nc = self.nc
    drain_inst = nc.sync.drain()
    wait_clock.add_sem_waits(
        drain_inst.ins, ScopedClock({None: tick_clock.global_clock})
    )
    si = drain_inst.ins.sync_info
    if si is not None and len(si.on_wait) > 0:
        waits = list(si.on_wait)
        si.on_wait = []
        for w in waits:
            nop = nc.sync.nop(nofuse=True)
            nop.ins.sync_info = mybir.SyncInfo(on_wait=[w], on_update=[])
    nc.all_engine_barrier()
    assert self.sems is not None
    popped = nc._tile_sem_poison_stack.pop()
    assert popped is self._sem_poison
    nc.clear_and_free_semaphores(list(self.sems.allocated().values()))
    nc.all_engine_barrier()


tile.TileContext._drain_and_barrier = _drain_and_barrier_split


def split_multi_waits(nc: "bass.Bass", max_waits: int = 1) -> None:
    k = 0
    for fn in nc.m.functions:
        for bb in fn.blocks:
            il = list(bb.instructions)
            out = []
            changed = False
            for inst in il:
                si = inst.sync_info
                if si is not None and len(si.on_wait) > max_waits:
                    waits = list(si.on_wait)
                    si.on_wait = waits[:max_waits]
                    for w in waits[max_waits:]:
                        nop = mybir.InstNoOp(
                            name=f"I-wsplit-{k}", ins=[], outs=[]
                        )
                        k += 1
                        nop.engine = inst.engine
                        nop.sync_info = mybir.SyncInfo(
                            on_wait=[w], on_update=[]
                        )
                        nc.register_instruction(nop, overwrite=True)
                        out.append(nop)
                        changed = True
                out.append(inst)
            if changed:
                bb.instructions = out


# ---------------------------------------------------------------------------
# Kernel
# ---------------------------------------------------------------------------

D = 128
P = 128
N_CORES = 8
NCHUNK = 4  # x row-quarters (int16 dma_gather index reach)


@dataclass(frozen=True)
class Cfg:
    n_nodes: int
    node_per_core: int
    seg_tiles: int  # tiles per (window, chunk) segment
    group_w: int  # windows per gather group
    gbufs: int = 2
    bf16: bool = False  # gather/one-hot/matmul in bf16 (PSUM stays f32)
    scratch: int = 65536  # SWDGE descriptor-ring carveout bytes
    bigout: bool = False  # accumulate output in SBUF, flush per group
    psum_bufs: int = 4
    v2: bool = False  # exact-count packed layout (build_v2/prep_v2)
    phases: int = 0  # >0: serialize gather/compute in this many phases
    indirect: bool = False  # gather via HWDGE indirect_dma_start (no Q7)
    out_bf16: bool = False  # write out in bf16 (host upcasts); halves
    # output DMA bytes; PSUM stays f32 so only one rounding is added

    @property
    def chunk_rows(self) -> int:
        return self.n_nodes // NCHUNK

    @property
    def W(self) -> int:
        return -(-self.node_per_core // P)

    @property
    def T(self) -> int:
        return NCHUNK * self.seg_tiles

    @property
    def NT(self) -> int:
        return self.W * self.T

    @property
    def region_cols(self) -> int:
        return self.W * self.seg_tiles

    @property
    def n_groups(self) -> int:
        assert self.W % self.group_w == 0
        return self.W // self.group_w

    @property
    def call_tiles(self) -> int:
        return self.group_w * self.seg_tiles

    @property
    def call_idxs(self) -> int:
        return self.call_tiles * P

    @property
    def idx_cols(self) -> int:
        return self.n_groups * (self.call_idxs // 16)

    @property
    def out_rows(self) -> int:
        return self.W * P

    @property
    def cap(self) -> int:
        return self.seg_tiles * P


CFG = Cfg(n_nodes=100000, node_per_core=12500, seg_tiles=2, group_w=7,
          gbufs=4, bf16=True, bigout=True, psum_bufs=8)


def build(cfg: Cfg, reps: int = 1) -> bass.Bass:
    from concourse.library_config import mlp
    from concourse.library_overlay import lower_extended_insts

    assert cfg.chunk_rows <= 32767
    mdt = mybir.dt.bfloat16 if cfg.bf16 else mybir.dt.float32
    nc = bass.Bass(num_swdge_queues=NCHUNK,
                   dynamic_dma_scratch_size=cfg.scratch)
    x = nc.declare_dram_parameter("x", [cfg.n_nodes, D], mdt,
                                  isOutput=False)
    if cfg.indirect:
        idxs = nc.declare_dram_parameter(
            "idxs", [P, cfg.NT], mybir.dt.int32, isOutput=False)
    else:
        idxs = nc.declare_dram_parameter(
            "idxs", [P, NCHUNK * cfg.idx_cols], mybir.dt.int16,
            isOutput=False)
    ohpos = nc.declare_dram_parameter("ohpos", [P, cfg.NT], mybir.dt.float32,
                                      isOutput=False)
    iota = nc.declare_dram_parameter("iota128", [P, P], mdt,
                                     isOutput=False)
    odt = mybir.dt.bfloat16 if cfg.out_bf16 else mybir.dt.float32
    out = nc.declare_dram_parameter("out", [cfg.out_rows, D],
                                    odt, isOutput=True)

    CT = cfg.call_tiles
    nc.gpsimd.load_library(mlp)
    with tile.TileContext(nc) as tc:
        with (
            tc.tile_pool(name="tabs", bufs=1) as tabs,
            tc.tile_pool(name="gbuf", bufs=cfg.gbufs) as gbuf,
            tc.tile_pool(name="ohb", bufs=4) as ohb,
            tc.tile_pool(name="psumb", bufs=cfg.psum_bufs,
                         space="PSUM") as psumb,
            tc.tile_pool(name="outb", bufs=4) as outb,
        ):
            idxs_sb = tabs.tile(
                [P, cfg.NT] if cfg.indirect else [P, NCHUNK * cfg.idx_cols],
                mybir.dt.int32 if cfg.indirect else mybir.dt.int16)
            ohpos_sb = tabs.tile([P, cfg.NT], mybir.dt.float32)
            iota_sb = tabs.tile([P, P], mdt)
            nc.sync.dma_start(out=idxs_sb[:], in_=idxs[:])
            nc.sync.dma_start(out=ohpos_sb[:], in_=ohpos[:])
            nc.sync.dma_start(out=iota_sb[:], in_=iota[:])
            outsb = (tabs.tile([P, cfg.W * D], odt,
                                name="outsb", tag="outsb")
                     if cfg.bigout else None)

            # one shared register for the gather count — a fresh to_reg per
            # call exhausts the Pool register file at this call count
            nidx_reg = nc.gpsimd.to_reg(cfg.call_idxs)

            for g in range(cfg.n_groups * reps):
                g = g % cfg.n_groups
                chts = []
                for r in range(NCHUNK):
                    ch = gbuf.tile([P, CT * D], mdt,
                                   tag=f"ch{r}")
                    if cfg.indirect:
                        i0 = r * cfg.region_cols + g * CT
                        nc.gpsimd.indirect_dma_start(
                            out=ch[:].rearrange("p (c d) -> p c d", c=CT),
                            out_offset=None,
                            in_=x[:, :],
                            in_offset=bass.IndirectOffsetOnAxis(
                                ap=idxs_sb[:, i0:i0 + CT],
                                axis=0,
                            ),
                        )
                        chts.append(ch)
                        continue
                    c0 = r * cfg.idx_cols + g * (cfg.call_idxs // 16)
                    nc.gpsimd.dma_gather(
                        ch[:].rearrange("p (c d) -> p c d", c=CT),
                        x[r * cfg.chunk_rows:(r + 1) * cfg.chunk_rows, :],
                        idxs_sb[:, c0:c0 + cfg.call_idxs // 16],
                        cfg.call_idxs,
                        nidx_reg,
                        D,
                        queue_num=r,
                        # single_packet coalesces the whole descriptor stream
                        # into one SDMA packet; beyond ~1K descriptors that
                        # wedges the DMA engine (HW hang). Multi-packet is
                        # required at this call size.
                        single_packet=False,
                    )
                    chts.append(ch)
                for wl in range(cfg.group_w):
                    w = g * cfg.group_w + wl
                    ps = psumb.tile([P, D], mybir.dt.float32, tag="ps")
                    k = 0
                    last = cfg.T - 1
                    for r in range(NCHUNK):
                        for i in range(cfg.seg_tiles):
                            tcol = (r * cfg.region_cols
                                    + cfg.seg_tiles * w + i)
                            lcol = cfg.seg_tiles * wl + i
                            oh = ohb.tile([P, P], mdt, tag="oh")
                            nc.vector.tensor_scalar(
                                out=oh[:],
                                in0=iota_sb[:],
                                scalar1=ohpos_sb[:, tcol:tcol + 1],
                                scalar2=None,
                                op0=mybir.AluOpType.is_equal,
                            )
                            nc.tensor.matmul(
                                ps[:],
                                lhsT=oh[:],
                                rhs=chts[r][:, lcol * D:(lcol + 1) * D],
                                start=(k == 0),
                                stop=(k == last),
                            )
                            k += 1
                    if cfg.bigout:
                        nc.scalar.copy(out=outsb[:, w * D:(w + 1) * D],
                                       in_=ps[:])
                        if wl == cfg.group_w - 1:
                            w0 = g * cfg.group_w
                            nc.sync.dma_start(
                                out=out[w0 * P:(w + 1) * P, :]
                                .rearrange("(w p) d -> p w d",
                                           w=cfg.group_w),
                                in_=outsb[:, w0 * D:(w + 1) * D]
                                .rearrange("p (w d) -> p w d",
                                           w=cfg.group_w),
                            )
                    else:
                        ob = outb.tile([P, D], odt, tag="ob")
                        nc.scalar.copy(out=ob[:], in_=ps[:])
                        nc.sync.dma_start(
                            out=out[w * P:(w + 1) * P, :], in_=ob[:]
                        )
    split_multi_waits(nc)
    lower_extended_insts(nc)
    return nc


# ---------------------------------------------------------------------------
# v2 layout: exact-count packed gather (no intra-segment padding)
#
# Edges are packed contiguously per (group, chunk) sorted by window; gather
# calls carry per-core valid counts in a Pool register (reg_load), so padding
# slots generate no DMA descriptors.  A tile may span up to two consecutive
# windows; the per-tile window list (union across all 8 cores, so one shared
# SPMD program works) drives one wide one-hot per <=2 windows: OH built over
# iota256 with compare value pos + 128*k, then one matmul per 128-col slice.
# ---------------------------------------------------------------------------


def prep_v2(x, edge_index, cfg: Cfg):
    row = np.asarray(edge_index[0]).astype(np.int64)
    col = np.asarray(edge_index[1]).astype(np.int64)
    mdt_np = mybir.dt.np(mybir.dt.bfloat16 if cfg.bf16 else mybir.dt.float32)
    xf = np.ascontiguousarray(np.asarray(x, dtype=np.float32).astype(mdt_np))
    NG, GW = cfg.n_groups, cfg.group_w

    cores = []
    wcounts = np.zeros((N_CORES, NG, NCHUNK, GW), np.int64)
    for c in range(N_CORES):
        lo = c * cfg.node_per_core
        m = (col >= lo) & (col < lo + cfg.node_per_core)
        lcol = col[m] - lo
        lrow = row[m]
        w = lcol >> 7
        pos = lcol & 127
        ck = lrow // cfg.chunk_rows
        lidx = lrow - ck * cfg.chunk_rows
        g = w // GW
        wl = w % GW
        order = np.lexsort((wl, ck, g))
        cores.append((g[order], ck[order], wl[order], pos[order],
                      lidx[order]))
        np.add.at(wcounts[c], (g[order], ck[order], wl[order]), 1)

    C = wcounts.sum(axis=3)  # [core, g, ck]
    if C.min() < 1:
        raise ValueError("v2 layout needs >=1 edge per (core, group, chunk)")
    T = -(-C.max(axis=0) // P)  # [g, ck] static tile counts

    winlists = []  # [g][ck] -> tuple per tile of sorted wl tuple
    for g in range(NG):
        row_l = []
        for ck in range(NCHUNK):
            tiles = [set() for _ in range(int(T[g, ck]))]
            for c in range(N_CORES):
                posn = 0
                for wl in range(GW):
                    n = int(wcounts[c, g, ck, wl])
                    if n == 0:
                        continue
                    for t in range(posn // P, (posn + n - 1) // P + 1):
                        tiles[t].add(wl)
                    posn += n
            row_l.append(tuple(tuple(sorted(s)) for s in tiles))
        winlists.append(tuple(row_l))
    Cmin = C.min(axis=0)  # [g, ck] min valid count across cores
    layout = (tuple(tuple(int(v) for v in T[g]) for g in range(NG)),
              tuple(winlists),
              tuple(tuple(int(v) for v in Cmin[g]) for g in range(NG)))

    # static oh-instruction schedule: per (g, ck, tile) split winlist into
    # chunks of <=2; each chunk is one ohpos column
    n_oh = 0
    for g in range(NG):
        for ck in range(NCHUNK):
            for wl_list in winlists[g][ck]:
                n_oh += max(1, -(-len(wl_list) // 2))
    Tmax = int(T.max())
    ci16_total = int(T.sum()) * 8

    in_maps = []
    for c in range(N_CORES):
        gs, cks, wls, poss, lidxs = cores[c]
        idxs = np.zeros((16, ci16_total), np.int16)
        ohpos = np.full((P, n_oh), -1.0, np.float32)
        counts = np.zeros((P, NG * NCHUNK), np.int32)
        base = 0
        ohcol = 0
        e0 = 0
        # edges are sorted by (g, ck, wl); walk segments in order
        for g in range(NG):
            for ck in range(NCHUNK):
                n = int(C[c, g, ck])
                tcount = int(T[g, ck])
                nslots = tcount * P
                sl = slice(e0, e0 + n)
                stream = np.full(nslots, -1, np.int16)
                stream[:n] = lidxs[sl]
                counts[:, g * NCHUNK + ck] = n
                idxs[:, base:base + tcount * 8] = (
                    stream.reshape(tcount * 8, 16).T
                )
                base += tcount * 8
                # oh columns for this segment
                wl_seg = wls[sl]
                pos_seg = poss[sl]
                for t, wl_list in enumerate(winlists[g][ck]):
                    s0, s1 = t * P, min((t + 1) * P, nslots)
                    nvals = max(0, min(s1, n) - s0)
                    lanes = np.arange(s0, s0 + nvals) - s0
                    for pair_i in range(max(1, -(-len(wl_list) // 2))):
                        pair = wl_list[2 * pair_i:2 * pair_i + 2]
                        if nvals > 0:
                            wl_t = wl_seg[s0:s0 + nvals]
                            pos_t = pos_seg[s0:s0 + nvals]
                            for k, wl in enumerate(pair):
                                mk = wl_t == wl
                                ohpos[lanes[mk], ohcol] = (
                                    pos_t[mk] + 128 * k
                                )
                        ohcol += 1
                e0 += n
        assert ohcol == n_oh and base == ci16_total
        it = np.tile(np.arange(256, dtype=np.float32), (P, 1)).astype(mdt_np)
        in_maps.append({"x": xf, "idxs": np.tile(idxs, (8, 1)),
                        "ohpos": ohpos, "iota256": it,
                        "counts": counts})
    return layout, in_maps


def build_v2(cfg: Cfg, layout, reps: int = 1) -> bass.Bass:
    from concourse.library_config import mlp
    from concourse.library_overlay import lower_extended_insts

    T, winlists, Cmin = layout
    NG, GW = cfg.n_groups, cfg.group_w
    Tmax = max(max(r) for r in T)
    ci16_total = sum(sum(r) for r in T) * 8
    n_oh = sum(max(1, -(-len(wl) // 2))
               for g in range(NG) for ck in range(NCHUNK)
               for wl in winlists[g][ck])
    mdt = mybir.dt.bfloat16 if cfg.bf16 else mybir.dt.float32

    nc = bass.Bass(num_swdge_queues=NCHUNK,
                   dynamic_dma_scratch_size=cfg.scratch)
    x = nc.declare_dram_parameter("x", [cfg.n_nodes, D], mdt, isOutput=False)
    idxs = nc.declare_dram_parameter("idxs", [P, ci16_total], mybir.dt.int16,
                                     isOutput=False)
    ohpos = nc.declare_dram_parameter("ohpos", [P, n_oh], mybir.dt.float32,
                                      isOutput=False)
    iota = nc.declare_dram_parameter("iota256", [P, 2 * P], mdt,
                                     isOutput=False)
    counts = nc.declare_dram_parameter("counts", [P, NG * NCHUNK],
                                       mybir.dt.int32, isOutput=False)
    out = nc.declare_dram_parameter("out", [cfg.out_rows, D],
                                    mybir.dt.float32, isOutput=True)

    nc.gpsimd.load_library(mlp)
    with tile.TileContext(nc) as tc:
        with (
            tc.tile_pool(name="tabs", bufs=1) as tabs,
            tc.tile_pool(name="gbuf", bufs=cfg.gbufs) as gbuf,
            tc.tile_pool(name="ohb", bufs=6) as ohb,
            tc.tile_pool(name="psumb", bufs=1, space="PSUM") as psumb,
        ):
            idxs_sb = tabs.tile([P, ci16_total], mybir.dt.int16)
            ohpos_sb = tabs.tile([P, n_oh], mybir.dt.float32)
            iota_sb = tabs.tile([P, 2 * P], mdt)
            counts_sb = tabs.tile([P, NG * NCHUNK], mybir.dt.int32)
            outsb = tabs.tile([P, cfg.W * D], mybir.dt.float32,
                              name="outsb", tag="outsb")
            nc.sync.dma_start(out=idxs_sb[:], in_=idxs[:])
            nc.sync.dma_start(out=ohpos_sb[:], in_=ohpos[:])
            nc.sync.dma_start(out=iota_sb[:], in_=iota[:])
            nc.sync.dma_start(out=counts_sb[:], in_=counts[:])

            cnt_reg = nc.gpsimd.to_reg(0)

            # static idx base offsets per (g, ck)
            bases = {}
            b = 0
            for g in range(NG):
                for ck in range(NCHUNK):
                    bases[(g, ck)] = b
                    b += T[g][ck] * 8

            # per-group matmul totals per window for start/stop flags
            mm_per_wl = []
            for g in range(NG):
                cnt = {wl: 0 for wl in range(GW)}
                for ck in range(NCHUNK):
                    for wl_list in winlists[g][ck]:
                        for wl in wl_list:
                            cnt[wl] += 1
                mm_per_wl.append(cnt)

            if cfg.phases:
                ppg = -(-NG // cfg.phases)  # groups per phase
                phase_groups = [list(range(p0, min(p0 + ppg, NG)))
                                for p0 in range(0, NG, ppg)]
            else:
                phase_groups = [[g] for g in range(NG)]
            all_chts = {}
            for pseq in range(len(phase_groups) * reps):
                glist = phase_groups[pseq % len(phase_groups)]
                for g in glist:
                    all_chts[g] = {}
                    chts = all_chts[g]
                    for ck in range(NCHUNK):
                        tcount = T[g][ck]
                        ch = gbuf.tile([P, Tmax * D], mdt, tag=f"ch{ck}")
                        c0 = bases[(g, ck)]
                        # zero the tiles past every core's valid count: the
                        # gather skips trailing -1 slots, and 0 * one-hot-0
                        # keeps them out of the sums (NaN-safe on fresh SBUF)
                        t0 = Cmin[g][ck] // P
                        nc.vector.memset(ch[:, t0 * D:tcount * D], 0.0)
                        nc.gpsimd.reg_load(
                            cnt_reg,
                            counts_sb[0:1,
                                      g * NCHUNK + ck:g * NCHUNK + ck + 1],
                        )
                        nc.gpsimd.dma_gather(
                            ch[:, :tcount * D].rearrange(
                                "p (c d) -> p c d", c=tcount),
                            x[ck * cfg.chunk_rows:
                              (ck + 1) * cfg.chunk_rows, :],
                            idxs_sb[:, c0:c0 + tcount * 8],
                            tcount * P,
                            cnt_reg,
                            D,
                            queue_num=ck,
                            single_packet=False,
                        )
                        chts[ck] = ch
                for g in (reversed(glist) if cfg.phases else glist):
                    chts = all_chts[g]
                    pss = {wl: psumb.tile([P, D], mybir.dt.float32,
                                          name=f"ps{wl}", tag=f"ps{wl}")
                           for wl in range(GW)}
                    seen = {wl: 0 for wl in range(GW)}
                    ohcol = sum(
                        max(1, -(-len(wl_list) // 2))
                        for gg in range(g)
                        for ck in range(NCHUNK)
                        for wl_list in winlists[gg][ck]
                    )
                    for ck in range(NCHUNK):
                        for t, wl_list in enumerate(winlists[g][ck]):
                            npair = max(1, -(-len(wl_list) // 2))
                            for pair_i in range(npair):
                                pair = wl_list[2 * pair_i:2 * pair_i + 2]
                                width = P * max(1, len(pair))
                                oh = ohb.tile([P, width], mdt,
                                              tag=f"oh{len(pair)}")
                                nc.vector.tensor_scalar(
                                    out=oh[:],
                                    in0=iota_sb[:, :width],
                                    scalar1=ohpos_sb[:, ohcol:ohcol + 1],
                                    scalar2=None,
                                    op0=mybir.AluOpType.is_equal,
                                )
                                for k, wl in enumerate(pair):
                                    seen[wl] += 1
                                    nc.tensor.matmul(
                                        pss[wl][:],
                                        lhsT=oh[:, k * P:(k + 1) * P],
                                        rhs=chts[ck][:, t * D:(t + 1) * D],
                                        start=(seen[wl] == 1),
                                        stop=(seen[wl] == mm_per_wl[g][wl]),
                                    )
                                ohcol += 1
                    for wl in range(GW):
                        w = g * GW + wl
                        nc.scalar.copy(out=outsb[:, w * D:(w + 1) * D],
                                       in_=pss[wl][:])
                    w0 = g * GW
                    nc.sync.dma_start(
                        out=out[w0 * P:(w0 + GW) * P, :]
                        .rearrange("(w p) d -> p w d", w=GW),
                        in_=outsb[:, w0 * D:(w0 + GW) * D]
                        .rearrange("p (w d) -> p w d", w=GW),
                    )
    split_multi_waits(nc)
    lower_extended_insts(nc)
    return nc


def prep_core(row, col, node_base, cfg: Cfg):
    """Slot assignment for one core. Returns (idxs int16, ohpos f32)."""
    lo, hi = node_base, node_base + cfg.node_per_core
    m = (col >= lo) & (col < hi)
    lcol = (col[m] - lo).astype(np.int64)
    lrow = row[m].astype(np.int64)

    w = lcol >> 7
    pos = lcol & 127
    ck = lrow // cfg.chunk_rows
    lidx = lrow - ck * cfg.chunk_rows

    key = w * NCHUNK + ck
    order = np.argsort(key, kind="stable")
    key_s = key[order]
    pos_s = pos[order]
    lidx_s = lidx[order]

    nseg = cfg.W * NCHUNK
    counts = np.bincount(key_s, minlength=nseg)
    if counts.max(initial=0) > cfg.cap:
        raise ValueError(
            f"segment overflow: {counts.max()} > {cfg.cap}"
        )
    starts = np.zeros(nseg, np.int64)
    np.cumsum(counts[:-1], out=starts[1:])
    rank = np.arange(len(key_s)) - starts[key_s]

    w_e = key_s // NCHUNK
    r_e = key_s % NCHUNK
    tcol = r_e * cfg.region_cols + w_e * cfg.seg_tiles + (rank >> 7)
    lane = rank & 127

    srcidx = np.zeros((P, cfg.NT), np.int16)
    ohpos = np.full((P, cfg.NT), -1.0, np.float32)
    srcidx[lane, tcol] = lidx_s
    ohpos[lane, tcol] = pos_s

    if cfg.indirect:
        # absolute int32 row index per slot, addressed [lane, tcol]
        # directly by the HWDGE indirect gather (no 16-wrap, no chunks)
        idxs = np.zeros((P, cfg.NT), np.int32)
        idxs[lane, tcol] = (r_e * cfg.chunk_rows + lidx_s).astype(np.int32)
        return idxs, ohpos

    # per (chunk, group) wrapped int16 index blocks: call order i = j*128+p,
    # wrapped in 16 partitions and replicated 8x (what the Q7 pairs read)
    idxs = np.zeros((P, NCHUNK * cfg.idx_cols), np.int16)
    CT = cfg.call_tiles
    ci16 = cfg.call_idxs // 16
    for r in range(NCHUNK):
        for g in range(cfg.n_groups):
            cols = r * cfg.region_cols + g * CT + np.arange(CT)
            flat = srcidx[:, cols].T.reshape(-1)
            wrapped = flat.reshape(ci16, 16).T
            c0 = r * cfg.idx_cols + g * ci16
            idxs[:, c0:c0 + ci16] = np.tile(wrapped, (8, 1))
    return idxs, ohpos


def prep_all(x, edge_index, cfg: Cfg):
    row = np.asarray(edge_index[0])
    col = np.asarray(edge_index[1])
    mdt_np = mybir.dt.np(mybir.dt.bfloat16 if cfg.bf16 else mybir.dt.float32)
    xf = np.ascontiguousarray(np.asarray(x, dtype=np.float32).astype(mdt_np))
    it = np.tile(np.arange(P, dtype=np.float32), (P, 1)).astype(mdt_np)
    in_maps = []
    for c in range(N_CORES):
        idxs, ohpos = prep_core(row, col, c * cfg.node_per_core, cfg)
        in_maps.append({"x": xf, "idxs": idxs, "ohpos": ohpos,
                        "iota128": it})
    return in_maps


class SpmdRunner:
    """PJRT SPMD runner for a prebuilt Bass module.

    Mirrors bass2jax.run_bass_via_pjrt but stages inputs with per-device
    device_put + make_array_from_single_device_arrays and reads outputs
    shard-by-shard: no host<->global-array slicing ops get compiled (this
    toolchain's penguin DataLocalityOpt rejects them for large arrays).
    """

    def __init__(self, nc: bass.Bass, n_cores: int = N_CORES):
        bass2jax.install_neuronx_cc_hook()
        self.nc = nc
        self.n_cores = n_cores
        pname = nc.partition_id_tensor.name if nc.partition_id_tensor else None
        self.partition_name = pname
        in_names, out_names, out_avals = [], [], []
        for alloc in nc.m.functions[0].allocations:
            if not isinstance(alloc, mybir.MemoryLocationSet):
                continue
            name = alloc.memorylocations[0].name
            if alloc.kind == "ExternalInput":
                if name != pname:
                    in_names.append(name)
            elif alloc.kind == "ExternalOutput":
                out_names.append(name)
                out_avals.append(
                    jax.core.ShapedArray(
                        tuple(alloc.tensor_shape), mybir.dt.np(alloc.dtype)
                    )
                )
        self.in_names = in_names
        self.out_names = out_names
        self.out_avals = out_avals
        self.devices = jax.devices()[:n_cores]
        self.mesh = Mesh(np.asarray(self.devices), ("core",))
        self.sharding = NamedSharding(self.mesh, PartitionSpec("core"))
        all_in_names = list(in_names) + list(out_names)
        if pname is not None:
            all_in_names.append(pname)

        def _body(*args):
            operands = list(args)
            if pname is not None:
                operands.append(bass2jax.partition_id_tensor())
            return tuple(
                bass2jax._bass_exec_p.bind(
                    *operands,
                    out_avals=tuple(out_avals),
                    in_names=tuple(all_in_names),
                    out_names=tuple(out_names),
                    lowering_input_output_aliases=(),
                    sim_require_finite=True,
                    sim_require_nnan=True,
                    nc=nc,
                )
            )

        n_args = len(in_names) + len(out_names)
        self.fn = jax.jit(
            shard_map(
                _body,
                mesh=self.mesh,
                in_specs=(PartitionSpec("core"),) * n_args,
                out_specs=(PartitionSpec("core"),) * len(out_names),
                check_rep=False,
            ),
            keep_unused=True,
        )

    def _global(self, per_core_arrays):
        shape = per_core_arrays[0].shape
        gshape = (self.n_cores * shape[0],) + tuple(shape[1:])
        bufs = [
            jax.device_put(a, d)
            for a, d in zip(per_core_arrays, self.devices)
        ]
        return jax.make_array_from_single_device_arrays(
            gshape, self.sharding, bufs
        )

    def stage(self, in_maps):
        args = [
            self._global([np.asarray(m[name]) for m in in_maps])
            for name in self.in_names
        ]
        args += [
            self._global(
                [np.zeros(av.shape, av.dtype) for _ in range(self.n_cores)]
            )
            for av in self.out_avals
        ]
        return args

    def run(self, args):
        outs = self.fn(*args)
        jax.block_until_ready(outs)
        return outs

    def to_numpy(self, outs):
        res = [dict() for _ in range(self.n_cores)]
        for i, name in enumerate(self.out_names):
            shards = sorted(
                outs[i].addressable_shards,
                key=lambda s: s.index[0].start or 0,
            )
            assert len(shards) == self.n_cores
            for c, s in enumerate(shards):
                res[c][name] = np.asarray(s.data)
        return res

    def __call__(self, in_maps):
        return self.to_numpy(self.run(self.stage(in_maps)))


_NC_CACHE = {}
_RUNNER_CACHE = {}


def _get_nc(cfg: Cfg) -> bass.Bass:
    nc = _NC_CACHE.get(cfg)
    if nc is None:
        nc = build(cfg)
        _NC_CACHE[cfg] = nc
    return nc


def _get_runner(cfg: Cfg) -> SpmdRunner:
    r = _RUNNER_CACHE.get(cfg)
    if r is None:
        r = SpmdRunner(_get_nc(cfg))
        _RUNNER_CACHE[cfg] = r
    return r


def _get_runner_v2(cfg: Cfg, layout) -> SpmdRunner:
    key = (cfg, layout)
    r = _RUNNER_CACHE.get(key)
    if r is None:
        nc = _NC_CACHE.get(key)
        if nc is None:
            nc = build_v2(cfg, layout)
            _NC_CACHE[key] = nc
        r = SpmdRunner(nc)
        _RUNNER_CACHE[key] = r
    return r


def _host_fallback(x, edge_index):
    out = np.zeros((x.shape[0], x.shape[1]), np.float32)
    np.add.at(
        out,
        np.asarray(edge_index[1], np.int64),
        np.asarray(x, np.float32)[np.asarray(edge_index[0], np.int64)],
    )
    return out


def kernel(x: np.ndarray, edge_index: np.ndarray) -> np.ndarray:
    x = np.asarray(x)
    edge_index = np.asarray(edge_index)
    if CFG.v2:
        try:
            layout, in_maps = prep_v2(x, edge_index, CFG)
            res = _get_runner_v2(CFG, layout)(in_maps)
        except ValueError:
            # Degenerate edge distribution (empty segment): host fallback
            # rather than returning garbage.
            return _host_fallback(x, edge_index)
        return np.concatenate(
            [res[c]["out"][: CFG.node_per_core] for c in range(N_CORES)]
        )
    try:
        in_maps = prep_all(x, edge_index, CFG)
    except ValueError:
        # Segment-capacity overflow (an edge distribution far from this
        # problem's uniform random graph): fall back to a host computation
        # rather than returning wrong results.
        return _host_fallback(x, edge_index)
    res = _get_runner(CFG)(in_maps)
    return np.concatenate(
        [res[c]["out"][: CFG.node_per_core] for c in range(N_CORES)]
    ).astype(np.float32)

